# revision 2
# baseline (speedup 1.0000x reference)
"""GQA attention layer (B=2, S=2048, H=2048, 16 q heads / 4 kv heads, RoPE, causal
mask) on 8 TRN2 NeuronCores.

Sharding: core c = (b, g) with b = c // 4 (batch), g = c % 4 (kv-head group).
Each core computes q-heads 4g..4g+3 and kv-head g for batch b:
  - QKV projections from x^T (bf16 matmuls, f32 PSUM accumulate)
  - RoPE on q/k (f32, host-precomputed cos/sin tables)
  - MM1 scores [q, k] -> mask -> fused exp+rowsum on ScalarE -> P (bf16)
  - P scaled by 1/rowsum, DMA-transposed -> P^T, MM2 -> ctx^T [d, q]
  - AllGather ctx^T across the 4 cores of the batch (groups [[0..3],[4..7]])
  - o-proj: each core computes its 512-column block of the output for all S.
Host reassembles [B, S, H] from the 8 [S, 512] column blocks.

The additive attention mask is handled generally: each 128x512 score tile is
classified at build time (from the actual mask input) as fully-masked (matmul
skipped), zero (no mask op), or mixed (a per-core mask pattern tile is added
pre-exp). For the causal mask this skips the upper triangle (~half the
attention FLOPs) and needs only 4 distinct pattern tiles.
"""
import math
import os
import sys

for _p in ("/opt/trn_rl_repo",):
    if _p not in sys.path and os.path.isdir(_p):
        sys.path.insert(0, _p)

import ml_dtypes
import numpy as np

from concourse import bacc, mybir, tile
from concourse.bass_utils import run_bass_kernel_spmd

BF16 = mybir.dt.bfloat16
F32 = mybir.dt.float32
EXP = mybir.ActivationFunctionType.Exp
ADD = mybir.AluOpType.add

B, S, H = 2, 2048, 2048
NH, NKV, HD = 16, 4, 128
GQ = NH // NKV            # q heads per core (4)
DL = GQ * HD              # local q width (512)
ET = H // 128             # e-tiles (16)
NSC = S // 512            # 512-wide s/k chunks (4)
NQT = S // 128            # 128-row q tiles (16)
NKT = S // 128            # 128-row k tiles (16)
ROPE_THETA = 10000.0
INV_SQRT_HD = 1.0 / math.sqrt(HD)
SQRT_HD = math.sqrt(HD)

SKIP, FREE = 0, 1         # vis codes; >=2 means pattern index (code - 2)

_cache = {}


def _classify_mask(mask):
    """mask: [B, 1, S, S] f32 additive. Returns (vis, pats) where
    vis[qt][kc] in {SKIP, FREE, idx+2} and pats[b] is [n_pat, 128, 512] f32
    (clipped, pre-multiplied by sqrt(HD))."""
    m = np.asarray(mask, dtype=np.float32).reshape(B, S, S)
    vis = [[FREE] * NSC for _ in range(NQT)]
    pat_ids = {}
    pats = [[] for _ in range(B)]
    for qt in range(NQT):
        for kc in range(NSC):
            blk = m[:, qt * 128:(qt + 1) * 128, kc * 512:(kc + 1) * 512]
            if np.all(blk <= -1e8):
                vis[qt][kc] = SKIP
            elif np.all(blk == 0.0):
                vis[qt][kc] = FREE
            else:
                clipped = np.maximum(blk, -90.0) * SQRT_HD
                key = clipped.tobytes()
                if key not in pat_ids:
                    pat_ids[key] = len(pats[0])
                    for b in range(B):
                        pats[b].append(clipped[b])
                vis[qt][kc] = pat_ids[key] + 2
    n_pat = len(pats[0])
    if n_pat == 0:
        pats_np = [np.zeros((1, 128, 512), np.float32) for _ in range(B)]
        n_pat = 1
    else:
        pats_np = [np.stack(p) for p in pats]
    return vis, pats_np, n_pat


def _build(vis, n_pat):
    nc = bacc.Bacc(None, target_bir_lowering=False, num_devices=8)

    xT_d = nc.dram_tensor("xT", [H, S], BF16, kind="ExternalInput")
    wqT_d = nc.dram_tensor("wqT", [H, DL], BF16, kind="ExternalInput")
    wkT_d = nc.dram_tensor("wkT", [H, HD], BF16, kind="ExternalInput")
    wvT_d = nc.dram_tensor("wvT", [H, HD], BF16, kind="ExternalInput")
    woT_d = nc.dram_tensor("woT", [H, DL], BF16, kind="ExternalInput")
    cosT_d = nc.dram_tensor("cosT", [HD, S], F32, kind="ExternalInput")
    sinT_d = nc.dram_tensor("sinT", [HD, S], F32, kind="ExternalInput")
    pats_d = nc.dram_tensor("pats", [n_pat, 128, 512], F32, kind="ExternalInput")
    out_d = nc.dram_tensor("out", [S, DL], F32, kind="ExternalOutput")

    # visible k-chunk list per q-tile; visible k-tile list per chunk of q-tiles
    viskc = [[kc for kc in range(NSC) if vis[qt][kc] != SKIP] for qt in range(NQT)]
    vis_kts = []
    for c in range(NSC):
        kts = sorted({kt for qs in range(4) for kc in viskc[4 * c + qs]
                      for kt in range(4 * kc, 4 * kc + 4)})
        vis_kts.append(kts)
    uniform = all(
        all(viskc[4 * c + qs] == viskc[4 * c] for qs in range(4)) for c in range(NSC)
    )

    with tile.TileContext(nc) as tc:
        with (
            tc.tile_pool(name="wp", bufs=1) as wp,
            tc.tile_pool(name="xp", bufs=2) as xp,
            tc.tile_pool(name="qk", bufs=1) as qk,
            tc.tile_pool(name="rp", bufs=2) as rp,
            tc.tile_pool(name="pp", bufs=1) as pp,
            tc.tile_pool(name="ptp", bufs=2) as ptp,
            tc.tile_pool(name="cp", bufs=2) as cp,
            tc.tile_pool(name="fp", bufs=1) as fp,
            tc.tile_pool(name="op", bufs=2) as op,
            tc.tile_pool(name="sp", bufs=6) as sp,
            tc.tile_pool(name="ps", bufs=1, space="PSUM") as psp,
            tc.tile_pool(name="dr", bufs=2, space="DRAM") as dr,
        ):
            # ---- persistent weights / tables ----
            wq_sb = [wp.tile([128, DL], BF16, name=f"wq{e}") for e in range(ET)]
            wk_sb = [wp.tile([128, HD], BF16, name=f"wk{e}") for e in range(ET)]
            wv_sb = [wp.tile([128, HD], BF16, name=f"wv{e}") for e in range(ET)]
            wo_sb = [wp.tile([128, DL], BF16, name=f"wo{e}") for e in range(ET)]
            for e in range(ET):
                r = slice(e * 128, (e + 1) * 128)
                nc.sync.dma_start(wq_sb[e][:], wqT_d[r, :])
                nc.sync.dma_start(wk_sb[e][:], wkT_d[r, :])
                nc.sync.dma_start(wv_sb[e][:], wvT_d[r, :])
                nc.sync.dma_start(wo_sb[e][:], woT_d[r, :])
            cos_sb = wp.tile([HD, S], F32, name="cos_sb")
            sin_sb = wp.tile([HD, S], F32, name="sin_sb")
            nc.sync.dma_start(cos_sb[:], cosT_d[:])
            nc.sync.dma_start(sin_sb[:], sinT_d[:])
            pat_sb = [wp.tile([128, 512], F32, name=f"pat{i}") for i in range(n_pat)]
            for i in range(n_pat):
                nc.sync.dma_start(pat_sb[i][:], pats_d[i])

            # ---- persistent activations ----
            qT = [qk.tile([HD, S], BF16, name=f"qT{h}") for h in range(GQ)]
            kT = qk.tile([HD, S], BF16, name="kT")
            v_all = qk.tile([128, NKT, HD], BF16, name="v_all")

            def rope(ps, out_slice, sc):
                cs = slice(sc * 512, (sc + 1) * 512)
                t1 = rp.tile([128, 512], F32, name="rope_t1")
                nc.vector.tensor_mul(t1[0:64, :], ps[64:128, :], sin_sb[0:64, cs])
                nc.vector.tensor_mul(t1[64:128, :], ps[0:64, :], sin_sb[64:128, cs])
                t2 = rp.tile([128, 512], F32, name="rope_t2")
                nc.vector.tensor_mul(t2[:], ps[:], cos_sb[:, cs])
                nc.vector.tensor_add(out_slice, t2[:], t1[:])

            # ---- phase 1: QKV projections ----
            for sc in range(NSC):
                cs = slice(sc * 512, (sc + 1) * 512)
                xts = []
                for e in range(ET):
                    t = xp.tile([128, 512], BF16, name=f"xts{e}")
                    nc.sync.dma_start(t[:], xT_d[e * 128:(e + 1) * 128, cs])
                    xts.append(t)

                def proj(lhs_slices):
                    ps = psp.tile([128, 512], F32, name="qkv_ps", bufs=2)
                    for e in range(ET):
                        nc.tensor.matmul(ps[:], lhs_slices[e], xts[e][:],
                                         start=(e == 0), stop=(e == ET - 1))
                    return ps

                for h in range(GQ):
                    ps = proj([wq_sb[e][:, h * 128:(h + 1) * 128] for e in range(ET)])
                    rope(ps, qT[h][:, cs], sc)
                ps = proj([wk_sb[e][:] for e in range(ET)])
                rope(ps, kT[:, cs], sc)
                ps = proj([wv_sb[e][:] for e in range(ET)])
                vt = rp.tile([128, 512], BF16, name="vt_tmp")
                nc.vector.tensor_copy(vt[:], ps[:])
                nc.sync.dma_start_transpose(v_all[:, sc * 4:(sc + 1) * 4, :], vt[:])

            # ---- phase 2: attention / AllGather / o-proj per 512-row q chunk ----
            for c in range(NSC):
                kts = vis_kts[c]
                ag_in = dr.tile([DL, 512], BF16, name="ag_in")
                for h in range(GQ):
                    PT = ptp.tile([128, NKT, 512], BF16, name="PT")
                    for qs in range(4):
                        qt = 4 * c + qs
                        vk = viskc[qt]
                        if not uniform:
                            for kt in kts:
                                if (kt // 4) not in vk:
                                    nc.vector.memset(
                                        PT[:, kt, qs * 128:(qs + 1) * 128], 0.0)
                        if not vk:
                            continue
                        P = pp.tile([128, S], BF16, name=f"P{qs}")
                        sums = sp.tile([128, 4], F32, name="sums")
                        for i, kc in enumerate(vk):
                            ps = psp.tile([128, 512], F32, name="mm1_ps", bufs=2)
                            nc.tensor.matmul(
                                ps[:], qT[h][:, qt * 128:(qt + 1) * 128],
                                kT[:, kc * 512:(kc + 1) * 512])
                            code = vis[qt][kc]
                            if code >= 2:
                                nc.vector.tensor_add(ps[:], ps[:], pat_sb[code - 2][:])
                            nc.scalar.activation(
                                P[:, kc * 512:(kc + 1) * 512], ps[:], EXP,
                                scale=INV_SQRT_HD, accum_out=sums[:, i:i + 1])
                        stot = sp.tile([128, 1], F32, name="stot")
                        if len(vk) > 1:
                            nc.vector.tensor_reduce(
                                stot[:], sums[:, 0:len(vk)],
                                axis=mybir.AxisListType.X, op=ADD)
                        else:
                            nc.vector.tensor_copy(stot[:], sums[:, 0:1])
                        recip = sp.tile([128, 1], F32, name="recip")
                        nc.vector.reciprocal(recip[:], stot[:])
                        # scale + transpose per contiguous run of visible chunks
                        runs = []
                        for kc in vk:
                            if runs and runs[-1][1] == kc:
                                runs[-1][1] = kc + 1
                            else:
                                runs.append([kc, kc + 1])
                        for kc0, kc1 in runs:
                            seg = slice(kc0 * 512, kc1 * 512)
                            nc.vector.tensor_scalar_mul(P[:, seg], P[:, seg],
                                                        recip[:, 0:1])
                            nc.sync.dma_start_transpose(
                                PT[:, 4 * kc0:4 * kc1, qs * 128:(qs + 1) * 128],
                                P[:, seg])
                    ctx_ps = psp.tile([128, 512], F32, name="ctx_ps", bufs=2)
                    for j, kt in enumerate(kts):
                        nc.tensor.matmul(ctx_ps[:], v_all[:, kt, :], PT[:, kt, :],
                                         start=(j == 0), stop=(j == len(kts) - 1))
                    ctx_sb = cp.tile([128, 512], BF16, name=f"ctx{h}")
                    nc.vector.tensor_copy(ctx_sb[:], ctx_ps[:])
                    nc.sync.dma_start(ag_in[h * 128:(h + 1) * 128, :], ctx_sb[:])

                ag_out = dr.tile([H, 512], BF16, name="ag_out")
                nc.gpsimd.collective_compute(
                    "AllGather", mybir.AluOpType.bypass,
                    replica_groups=[[0, 1, 2, 3], [4, 5, 6, 7]],
                    ins=[ag_in[:].opt()], outs=[ag_out[:].opt()])

                ctxF = []
                for i in range(ET):
                    t = fp.tile([128, 512], BF16, name=f"ctxF{i}")
                    nc.sync.dma_start(t[:], ag_out[i * 128:(i + 1) * 128, :])
                    ctxF.append(t)
                for qs in range(4):
                    ops = psp.tile([128, 512], F32, name="oproj_ps", bufs=2)
                    for i in range(ET):
                        nc.tensor.matmul(ops[:],
                                         ctxF[i][:, qs * 128:(qs + 1) * 128],
                                         wo_sb[i][:],
                                         start=(i == 0), stop=(i == ET - 1))
                    osb = op.tile([128, 512], F32, name="osb")
                    nc.scalar.copy(osb[:], ops[:])
                    r0 = c * 512 + qs * 128
                    nc.sync.dma_start(out_d[r0:r0 + 128, :], osb[:])

    nc.compile()
    return nc


def kernel(hidden_states, wq, wk, wv, wo, attention_mask, position_ids):
    hidden_states = np.asarray(hidden_states, dtype=np.float32)
    wq = np.asarray(wq, dtype=np.float32)
    wk = np.asarray(wk, dtype=np.float32)
    wv = np.asarray(wv, dtype=np.float32)
    wo = np.asarray(wo, dtype=np.float32)
    pos = np.asarray(position_ids)

    vis, pats, n_pat = _classify_mask(attention_mask)
    key = (tuple(tuple(r) for r in vis), n_pat)
    if key not in _cache:
        _cache[key] = _build(vis, n_pat)
    nc = _cache[key]

    # RoPE tables per batch: cosT/sinT [HD, S]; sinT sign-folded (-sin for d<64)
    inv_freq = 1.0 / (ROPE_THETA ** (np.arange(0, HD, 2, dtype=np.float32) / HD))
    cosT = np.empty((B, HD, S), np.float32)
    sinT = np.empty((B, HD, S), np.float32)
    for b in range(B):
        freqs = pos[b].astype(np.float32)[None, :] * inv_freq[:, None]  # [64, S]
        cosT[b] = np.concatenate([np.cos(freqs)] * 2, axis=0)
        sn = np.sin(freqs)
        sinT[b] = np.concatenate([-sn, sn], axis=0)

    bf = ml_dtypes.bfloat16
    xT = [np.ascontiguousarray(hidden_states[b].T).astype(bf) for b in range(B)]
    in_maps = []
    for c in range(8):
        b, g = c // 4, c % 4
        in_maps.append({
            "xT": xT[b],
            "wqT": np.ascontiguousarray(wq[g * DL:(g + 1) * DL, :].T).astype(bf),
            "wkT": np.ascontiguousarray(wk[g * HD:(g + 1) * HD, :].T).astype(bf),
            "wvT": np.ascontiguousarray(wv[g * HD:(g + 1) * HD, :].T).astype(bf),
            "woT": np.ascontiguousarray(wo[g * DL:(g + 1) * DL, :].T).astype(bf),
            "cosT": cosT[b],
            "sinT": sinT[b],
            "pats": pats[b],
        })

    res = run_bass_kernel_spmd(nc, in_maps, core_ids=list(range(8))).results
    out = np.empty((B, S, H), np.float32)
    for c in range(8):
        b, g = c // 4, c % 4
        out[b, :, g * DL:(g + 1) * DL] = res[c]["out"]
    return out


# revision 3
# speedup vs baseline: 1.0413x; 1.0413x over previous
"""GQA attention layer (B=2, S=2048, H=2048, 16 q heads / 4 kv heads, RoPE, causal
mask) on 8 TRN2 NeuronCores.

Sharding: core c = (b, g) with b = c // 4 (batch), g = c % 4 (kv-head group).
Each core computes q-heads 4g..4g+3 and kv-head g for batch b:
  - QKV projections from x^T (bf16 matmuls, f32 PSUM accumulate)
  - RoPE on q/k (f32, host-precomputed cos/sin tables)
  - MM1 scores [q, k] -> mask -> fused exp+rowsum on ScalarE -> P (bf16)
  - P scaled by 1/rowsum, DMA-transposed -> P^T, MM2 -> ctx^T [d, q]
  - AllGather ctx^T across the 4 cores of the batch (groups [[0..3],[4..7]])
  - o-proj: each core computes its 512-column block of the output for all S.
Host reassembles [B, S, H] from the 8 [S, 512] column blocks.

The additive attention mask is handled generally: each 128x512 score tile is
classified at build time (from the actual mask input) as fully-masked (matmul
skipped), zero (no mask op), or mixed (a per-core mask pattern tile is added
pre-exp). For the causal mask this skips the upper triangle (~half the
attention FLOPs) and needs only 4 distinct pattern tiles.

Emission is software-pipelined at 512-row chunk granularity
(qkv(0) qkv(1) attn(0) qkv(2) attn(1) oproj(0) qkv(3) attn(2) oproj(1)
attn(3) oproj(2) oproj(3)) so the TensorEngine always has dense matmul work
while ScalarE exp / AllGather latency is in flight.
"""
import math
import os
import sys

for _p in ("/opt/trn_rl_repo",):
    if _p not in sys.path and os.path.isdir(_p):
        sys.path.insert(0, _p)

import ml_dtypes
import numpy as np

from concourse import bacc, mybir, tile
from concourse.bass_utils import run_bass_kernel_spmd

BF16 = mybir.dt.bfloat16
F32 = mybir.dt.float32
EXP = mybir.ActivationFunctionType.Exp
ADD = mybir.AluOpType.add

B, S, H = 2, 2048, 2048
NH, NKV, HD = 16, 4, 128
GQ = NH // NKV            # q heads per core (4)
DL = GQ * HD              # local q width (512)
ET = H // 128             # e-tiles (16)
NSC = S // 512            # 512-wide s/k chunks (4)
NQT = S // 128            # 128-row q tiles (16)
NKT = S // 128            # 128-row k tiles (16)
ROPE_THETA = 10000.0
INV_SQRT_HD = 1.0 / math.sqrt(HD)
SQRT_HD = math.sqrt(HD)

SKIP, FREE = 0, 1         # vis codes; >=2 means pattern index (code - 2)

_cache = {}


def _classify_mask(mask):
    """mask: [B, 1, S, S] f32 additive. Returns (vis, pats) where
    vis[qt][kc] in {SKIP, FREE, idx+2} and pats[b] is [n_pat, 128, 512] f32
    (clipped, pre-multiplied by sqrt(HD))."""
    m = np.asarray(mask, dtype=np.float32).reshape(B, S, S)
    vis = [[FREE] * NSC for _ in range(NQT)]
    pat_ids = {}
    pats = [[] for _ in range(B)]
    for qt in range(NQT):
        for kc in range(NSC):
            blk = m[:, qt * 128:(qt + 1) * 128, kc * 512:(kc + 1) * 512]
            if np.all(blk <= -1e8):
                vis[qt][kc] = SKIP
            elif np.all(blk == 0.0):
                vis[qt][kc] = FREE
            else:
                clipped = np.maximum(blk, -90.0) * SQRT_HD
                key = clipped.tobytes()
                if key not in pat_ids:
                    pat_ids[key] = len(pats[0])
                    for b in range(B):
                        pats[b].append(clipped[b])
                vis[qt][kc] = pat_ids[key] + 2
    n_pat = len(pats[0])
    if n_pat == 0:
        pats_np = [np.zeros((1, 128, 512), np.float32) for _ in range(B)]
        n_pat = 1
    else:
        pats_np = [np.stack(p) for p in pats]
    return vis, pats_np, n_pat


def _build(vis, n_pat):
    nc = bacc.Bacc(None, target_bir_lowering=False, num_devices=8)

    xT_d = nc.dram_tensor("xT", [H, S], BF16, kind="ExternalInput")
    wqT_d = nc.dram_tensor("wqT", [H, DL], BF16, kind="ExternalInput")
    wkT_d = nc.dram_tensor("wkT", [H, HD], BF16, kind="ExternalInput")
    wvT_d = nc.dram_tensor("wvT", [H, HD], BF16, kind="ExternalInput")
    woT_d = nc.dram_tensor("woT", [H, DL], BF16, kind="ExternalInput")
    cosT_d = nc.dram_tensor("cosT", [HD, S], F32, kind="ExternalInput")
    sinT_d = nc.dram_tensor("sinT", [HD, S], F32, kind="ExternalInput")
    pats_d = nc.dram_tensor("pats", [n_pat, 128, 512], F32, kind="ExternalInput")
    out_d = nc.dram_tensor("out", [S, DL], F32, kind="ExternalOutput")

    viskc = [[kc for kc in range(NSC) if vis[qt][kc] != SKIP] for qt in range(NQT)]
    vis_kts = []
    for c in range(NSC):
        kts = sorted({kt for qs in range(4) for kc in viskc[4 * c + qs]
                      for kt in range(4 * kc, 4 * kc + 4)})
        vis_kts.append(kts)
    uniform = all(
        all(viskc[4 * c + qs] == viskc[4 * c] for qs in range(4)) for c in range(NSC)
    )

    with tile.TileContext(nc) as tc:
        with (
            tc.tile_pool(name="wp", bufs=1) as wp,
            tc.tile_pool(name="xp", bufs=2) as xp,
            tc.tile_pool(name="qk", bufs=1) as qk,
            tc.tile_pool(name="rp", bufs=2) as rp,
            tc.tile_pool(name="pp", bufs=1) as pp,
            tc.tile_pool(name="ptp", bufs=2) as ptp,
            tc.tile_pool(name="cp", bufs=2) as cp,
            tc.tile_pool(name="fp", bufs=1) as fp,
            tc.tile_pool(name="op", bufs=2) as op,
            tc.tile_pool(name="sp", bufs=6) as sp,
            tc.tile_pool(name="ps", bufs=1, space="PSUM") as psp,
            tc.tile_pool(name="dr", bufs=2, space="DRAM") as dr,
        ):
            # ---- weights / tables needed by phase 1 ----
            wq_sb = [wp.tile([128, DL], BF16, name=f"wq{e}") for e in range(ET)]
            wk_sb = [wp.tile([128, HD], BF16, name=f"wk{e}") for e in range(ET)]
            wv_sb = [wp.tile([128, HD], BF16, name=f"wv{e}") for e in range(ET)]
            for e in range(ET):
                r = slice(e * 128, (e + 1) * 128)
                nc.sync.dma_start(wq_sb[e][:], wqT_d[r, :])
                nc.sync.dma_start(wk_sb[e][:], wkT_d[r, :])
                nc.sync.dma_start(wv_sb[e][:], wvT_d[r, :])
            cos_sb = wp.tile([HD, S], F32, name="cos_sb")
            sin_sb = wp.tile([HD, S], F32, name="sin_sb")
            nc.sync.dma_start(cos_sb[:], cosT_d[:])
            nc.sync.dma_start(sin_sb[:], sinT_d[:])

            # ---- persistent activations ----
            qT = [qk.tile([HD, S], BF16, name=f"qT{h}") for h in range(GQ)]
            kT = qk.tile([HD, S], BF16, name="kT")
            v_all = qk.tile([128, NKT, HD], BF16, name="v_all")
            wo_sb = [wp.tile([128, DL], BF16, name=f"wo{e}") for e in range(ET)]
            pat_sb = [wp.tile([128, 512], F32, name=f"pat{i}") for i in range(n_pat)]

            def rope(ps, out_slice, sc):
                cs = slice(sc * 512, (sc + 1) * 512)
                t1 = rp.tile([128, 512], F32, name="rope_t1")
                nc.vector.tensor_mul(t1[0:64, :], ps[64:128, :], sin_sb[0:64, cs])
                nc.vector.tensor_mul(t1[64:128, :], ps[0:64, :], sin_sb[64:128, cs])
                t2 = rp.tile([128, 512], F32, name="rope_t2")
                nc.vector.tensor_mul(t2[:], ps[:], cos_sb[:, cs])
                nc.vector.tensor_add(out_slice, t2[:], t1[:])

            def emit_qkv(sc):
                cs = slice(sc * 512, (sc + 1) * 512)
                xts = []
                for e in range(ET):
                    t = xp.tile([128, 512], BF16, name=f"xts{e}")
                    nc.sync.dma_start(t[:], xT_d[e * 128:(e + 1) * 128, cs])
                    xts.append(t)

                def proj(lhs_slices):
                    ps = psp.tile([128, 512], F32, name="qkv_ps", bufs=2)
                    for e in range(ET):
                        nc.tensor.matmul(ps[:], lhs_slices[e], xts[e][:],
                                         start=(e == 0), stop=(e == ET - 1))
                    return ps

                for h in range(GQ):
                    ps = proj([wq_sb[e][:, h * 128:(h + 1) * 128] for e in range(ET)])
                    rope(ps, qT[h][:, cs], sc)
                ps = proj([wk_sb[e][:] for e in range(ET)])
                rope(ps, kT[:, cs], sc)
                ps = proj([wv_sb[e][:] for e in range(ET)])
                vt = rp.tile([128, 512], BF16, name="vt_tmp")
                nc.vector.tensor_copy(vt[:], ps[:])
                nc.sync.dma_start_transpose(v_all[:, sc * 4:(sc + 1) * 4, :], vt[:])

            def emit_attn(c):
                """MM1 + exp + scale + transpose + MM2 + AllGather for q chunk c."""
                kts = vis_kts[c]
                ag_in = dr.tile([DL, 512], BF16, name="ag_in")
                for h in range(GQ):
                    PT = ptp.tile([128, NKT, 512], BF16, name="PT")
                    for qs in range(4):
                        qt = 4 * c + qs
                        vk = viskc[qt]
                        if not uniform:
                            for kt in kts:
                                if (kt // 4) not in vk:
                                    nc.vector.memset(
                                        PT[:, kt, qs * 128:(qs + 1) * 128], 0.0)
                        if not vk:
                            continue
                        P = pp.tile([128, S], BF16, name=f"P{qs}")
                        sums = sp.tile([128, 4], F32, name="sums")
                        for i, kc in enumerate(vk):
                            ps = psp.tile([128, 512], F32, name="mm1_ps", bufs=3)
                            nc.tensor.matmul(
                                ps[:], qT[h][:, qt * 128:(qt + 1) * 128],
                                kT[:, kc * 512:(kc + 1) * 512])
                            code = vis[qt][kc]
                            if code >= 2:
                                nc.vector.tensor_add(ps[:], ps[:], pat_sb[code - 2][:])
                            nc.scalar.activation(
                                P[:, kc * 512:(kc + 1) * 512], ps[:], EXP,
                                scale=INV_SQRT_HD, accum_out=sums[:, i:i + 1])
                        stot = sp.tile([128, 1], F32, name="stot")
                        if len(vk) > 1:
                            nc.vector.tensor_reduce(
                                stot[:], sums[:, 0:len(vk)],
                                axis=mybir.AxisListType.X, op=ADD)
                        else:
                            nc.vector.tensor_copy(stot[:], sums[:, 0:1])
                        recip = sp.tile([128, 1], F32, name="recip")
                        nc.vector.reciprocal(recip[:], stot[:])
                        runs = []
                        for kc in vk:
                            if runs and runs[-1][1] == kc:
                                runs[-1][1] = kc + 1
                            else:
                                runs.append([kc, kc + 1])
                        for kc0, kc1 in runs:
                            seg = slice(kc0 * 512, kc1 * 512)
                            nc.vector.tensor_scalar_mul(P[:, seg], P[:, seg],
                                                        recip[:, 0:1])
                            nc.sync.dma_start_transpose(
                                PT[:, 4 * kc0:4 * kc1, qs * 128:(qs + 1) * 128],
                                P[:, seg])
                    ctx_ps = psp.tile([128, 512], F32, name="ctx_ps", bufs=2)
                    for j, kt in enumerate(kts):
                        nc.tensor.matmul(ctx_ps[:], v_all[:, kt, :], PT[:, kt, :],
                                         start=(j == 0), stop=(j == len(kts) - 1))
                    ctx_sb = cp.tile([128, 512], BF16, name=f"ctx{h}")
                    nc.vector.tensor_copy(ctx_sb[:], ctx_ps[:])
                    nc.sync.dma_start(ag_in[h * 128:(h + 1) * 128, :], ctx_sb[:])

                ag_out = dr.tile([H, 512], BF16, name="ag_out")
                nc.gpsimd.collective_compute(
                    "AllGather", mybir.AluOpType.bypass,
                    replica_groups=[[0, 1, 2, 3], [4, 5, 6, 7]],
                    ins=[ag_in[:].opt()], outs=[ag_out[:].opt()])
                return ag_out

            def emit_oproj(c, ag_out):
                ctxF = []
                for i in range(ET):
                    t = fp.tile([128, 512], BF16, name=f"ctxF{i}")
                    nc.sync.dma_start(t[:], ag_out[i * 128:(i + 1) * 128, :])
                    ctxF.append(t)
                for qs in range(4):
                    ops = psp.tile([128, 512], F32, name="oproj_ps", bufs=1)
                    for i in range(ET):
                        nc.tensor.matmul(ops[:],
                                         ctxF[i][:, qs * 128:(qs + 1) * 128],
                                         wo_sb[i][:],
                                         start=(i == 0), stop=(i == ET - 1))
                    osb = op.tile([128, 512], F32, name="osb")
                    nc.vector.tensor_copy(osb[:], ops[:])
                    r0 = c * 512 + qs * 128
                    nc.sync.dma_start(out_d[r0:r0 + 128, :], osb[:])

            def emit_late_loads():
                for i in range(n_pat):
                    nc.sync.dma_start(pat_sb[i][:], pats_d[i])
                for e in range(ET):
                    nc.sync.dma_start(wo_sb[e][:], woT_d[e * 128:(e + 1) * 128, :])

            # ---- pipelined emission ----
            ag_outs = {}
            emit_qkv(0)
            emit_late_loads()
            emit_qkv(1)
            ag_outs[0] = emit_attn(0)
            emit_qkv(2)
            ag_outs[1] = emit_attn(1)
            emit_oproj(0, ag_outs[0])
            emit_qkv(3)
            ag_outs[2] = emit_attn(2)
            emit_oproj(1, ag_outs[1])
            ag_outs[3] = emit_attn(3)
            emit_oproj(2, ag_outs[2])
            emit_oproj(3, ag_outs[3])

    nc.compile()
    return nc


def kernel(hidden_states, wq, wk, wv, wo, attention_mask, position_ids):
    hidden_states = np.asarray(hidden_states, dtype=np.float32)
    wq = np.asarray(wq, dtype=np.float32)
    wk = np.asarray(wk, dtype=np.float32)
    wv = np.asarray(wv, dtype=np.float32)
    wo = np.asarray(wo, dtype=np.float32)
    pos = np.asarray(position_ids)

    vis, pats, n_pat = _classify_mask(attention_mask)
    key = (tuple(tuple(r) for r in vis), n_pat)
    if key not in _cache:
        _cache[key] = _build(vis, n_pat)
    nc = _cache[key]

    # RoPE tables per batch: cosT/sinT [HD, S]; sinT sign-folded (-sin for d<64)
    inv_freq = 1.0 / (ROPE_THETA ** (np.arange(0, HD, 2, dtype=np.float32) / HD))
    cosT = np.empty((B, HD, S), np.float32)
    sinT = np.empty((B, HD, S), np.float32)
    for b in range(B):
        freqs = pos[b].astype(np.float32)[None, :] * inv_freq[:, None]  # [64, S]
        cosT[b] = np.concatenate([np.cos(freqs)] * 2, axis=0)
        sn = np.sin(freqs)
        sinT[b] = np.concatenate([-sn, sn], axis=0)

    bf = ml_dtypes.bfloat16
    xT = [np.ascontiguousarray(hidden_states[b].T).astype(bf) for b in range(B)]
    in_maps = []
    for c in range(8):
        b, g = c // 4, c % 4
        in_maps.append({
            "xT": xT[b],
            "wqT": np.ascontiguousarray(wq[g * DL:(g + 1) * DL, :].T).astype(bf),
            "wkT": np.ascontiguousarray(wk[g * HD:(g + 1) * HD, :].T).astype(bf),
            "wvT": np.ascontiguousarray(wv[g * HD:(g + 1) * HD, :].T).astype(bf),
            "woT": np.ascontiguousarray(wo[g * DL:(g + 1) * DL, :].T).astype(bf),
            "cosT": cosT[b],
            "sinT": sinT[b],
            "pats": pats[b],
        })

    res = run_bass_kernel_spmd(nc, in_maps, core_ids=list(range(8))).results
    out = np.empty((B, S, H), np.float32)
    for c in range(8):
        b, g = c // 4, c % 4
        out[b, :, g * DL:(g + 1) * DL] = res[c]["out"]
    return out


# revision 6
# speedup vs baseline: 1.0969x; 1.0533x over previous
"""GQA attention layer (B=2, S=2048, H=2048, 16 q heads / 4 kv heads, RoPE, causal
mask) on 8 TRN2 NeuronCores.

Sharding: core c = (b, g) with b = c // 4 (batch), g = c % 4 (kv-head group).
Each core computes q-heads 4g..4g+3 and kv-head g for batch b:
  - QKV projections from x^T (bf16 matmuls, f32 PSUM accumulate)
  - RoPE on q/k (f32, host-precomputed cos/sin tables)
  - MM1 scores [q, k] -> mask -> fused exp+rowsum on ScalarE -> P (bf16)
  - P scaled by 1/rowsum, DMA-transposed -> P^T, MM2 -> ctx^T [d, q]
  - AllGather ctx^T across the 4 cores of the batch (groups [[0..3],[4..7]])
  - o-proj: each core computes its 512-column block of the output for all S.
Host reassembles [B, S, H] from the 8 [S, 512] column blocks.

The additive attention mask is handled generally: each 128x512 score tile is
classified at build time (from the actual mask input) as fully-masked (matmul
skipped), zero (no mask op), or mixed (a per-core mask pattern tile is added
pre-exp). For the causal mask this skips the upper triangle (~half the
attention FLOPs) and needs only 4 distinct pattern tiles.

The attention phase alone is exp(ScalarE)-bound, which starves TensorE and
triggers HAM re-throttling. So emission interleaves "filler" matmul groups
(QKV projection of the next chunk, o-proj of the previous chunk) between the
dependent MM1/MM2 groups of each attention head, keeping TensorE dense.
"""
import math
import os
import sys

for _p in ("/opt/trn_rl_repo",):
    if _p not in sys.path and os.path.isdir(_p):
        sys.path.insert(0, _p)

import ml_dtypes
import numpy as np

from concourse import bacc, mybir, tile
from concourse.bass_utils import run_bass_kernel_spmd

BF16 = mybir.dt.bfloat16
F32 = mybir.dt.float32
EXP = mybir.ActivationFunctionType.Exp
ADD = mybir.AluOpType.add

B, S, H = 2, 2048, 2048
NH, NKV, HD = 16, 4, 128
GQ = NH // NKV            # q heads per core (4)
DL = GQ * HD              # local q width (512)
ET = H // 128             # e-tiles (16)
NSC = S // 512            # 512-wide s/k chunks (4)
NQT = S // 128            # 128-row q tiles (16)
NKT = S // 128            # 128-row k tiles (16)
ROPE_THETA = 10000.0
INV_SQRT_HD = 1.0 / math.sqrt(HD)
SQRT_HD = math.sqrt(HD)

SKIP, FREE = 0, 1         # vis codes; >=2 means pattern index (code - 2)

_cache = {}


def _classify_mask(mask):
    """mask: [B, 1, S, S] f32 additive. Returns (vis, pats) where
    vis[qt][kc] in {SKIP, FREE, idx+2} and pats[b] is [n_pat, 128, 512] f32
    (clipped, pre-multiplied by sqrt(HD))."""
    m = np.asarray(mask, dtype=np.float32).reshape(B, S, S)
    vis = [[FREE] * NSC for _ in range(NQT)]
    pat_ids = {}
    pats = [[] for _ in range(B)]
    for qt in range(NQT):
        for kc in range(NSC):
            blk = m[:, qt * 128:(qt + 1) * 128, kc * 512:(kc + 1) * 512]
            if np.all(blk <= -1e8):
                vis[qt][kc] = SKIP
            elif np.all(blk == 0.0):
                vis[qt][kc] = FREE
            else:
                clipped = np.maximum(blk, -90.0) * SQRT_HD
                key = clipped.tobytes()
                if key not in pat_ids:
                    pat_ids[key] = len(pats[0])
                    for b in range(B):
                        pats[b].append(clipped[b])
                vis[qt][kc] = pat_ids[key] + 2
    n_pat = len(pats[0])
    if n_pat == 0:
        pats_np = [np.zeros((1, 128, 512), np.float32) for _ in range(B)]
        n_pat = 1
    else:
        pats_np = [np.stack(p) for p in pats]
    return vis, pats_np, n_pat


def _build(vis, n_pat):
    nc = bacc.Bacc(None, target_bir_lowering=False, num_devices=8)

    xT_d = nc.dram_tensor("xT", [H, S], BF16, kind="ExternalInput")
    wqT_d = nc.dram_tensor("wqT", [H, DL], BF16, kind="ExternalInput")
    wkT_d = nc.dram_tensor("wkT", [H, HD], BF16, kind="ExternalInput")
    wvT_d = nc.dram_tensor("wvT", [H, HD], BF16, kind="ExternalInput")
    woT_d = nc.dram_tensor("woT", [H, DL], BF16, kind="ExternalInput")
    cosT_d = nc.dram_tensor("cosT", [HD, S], F32, kind="ExternalInput")
    sinT_d = nc.dram_tensor("sinT", [HD, S], F32, kind="ExternalInput")
    pats_d = nc.dram_tensor("pats", [n_pat, 128, 512], F32, kind="ExternalInput")
    out_d = nc.dram_tensor("out", [S, DL], F32, kind="ExternalOutput")

    viskc = [[kc for kc in range(NSC) if vis[qt][kc] != SKIP] for qt in range(NQT)]
    vis_kts = []
    for c in range(NSC):
        kts = sorted({kt for qs in range(4) for kc in viskc[4 * c + qs]
                      for kt in range(4 * kc, 4 * kc + 4)})
        vis_kts.append(kts)
    uniform = all(
        all(viskc[4 * c + qs] == viskc[4 * c] for qs in range(4)) for c in range(NSC)
    )

    with tile.TileContext(nc) as tc:
        with (
            tc.tile_pool(name="wp", bufs=1) as wp,
            tc.tile_pool(name="xp", bufs=2) as xp,
            tc.tile_pool(name="qk", bufs=1) as qk,
            tc.tile_pool(name="rp", bufs=2) as rp,
            tc.tile_pool(name="pp", bufs=1) as pp,
            tc.tile_pool(name="ptp", bufs=2) as ptp,
            tc.tile_pool(name="cp", bufs=2) as cp,
            tc.tile_pool(name="fp", bufs=1) as fp,
            tc.tile_pool(name="op", bufs=2) as op,
            tc.tile_pool(name="sp", bufs=6) as sp,
            tc.tile_pool(name="ps", bufs=1, space="PSUM") as psp,
            tc.tile_pool(name="dr", bufs=2, space="DRAM") as dr,
        ):
            # ---- weights / tables needed by phase 1 (split across both
            # HWDGE trigger engines so loads run in parallel) ----
            wq_sb = [wp.tile([128, DL], BF16, name=f"wq{e}") for e in range(ET)]
            wk_sb = [wp.tile([128, HD], BF16, name=f"wk{e}") for e in range(ET)]
            wv_sb = [wp.tile([128, HD], BF16, name=f"wv{e}") for e in range(ET)]
            for e in range(ET):
                r = slice(e * 128, (e + 1) * 128)
                nc.sync.dma_start(wq_sb[e][:], wqT_d[r, :])
                nc.scalar.dma_start(wk_sb[e][:], wkT_d[r, :])
                nc.scalar.dma_start(wv_sb[e][:], wvT_d[r, :])
            cos_sb = wp.tile([HD, S], F32, name="cos_sb")
            sin_sb = wp.tile([HD, S], F32, name="sin_sb")
            nc.scalar.dma_start(cos_sb[:], cosT_d[:])
            nc.scalar.dma_start(sin_sb[:], sinT_d[:])

            # ---- persistent activations ----
            qT = [qk.tile([HD, S], BF16, name=f"qT{h}") for h in range(GQ)]
            kT = qk.tile([HD, S], BF16, name="kT")
            v_all = qk.tile([128, NKT, HD], BF16, name="v_all")
            wo_sb = [wp.tile([128, DL], BF16, name=f"wo{e}") for e in range(ET)]
            pat_sb = [wp.tile([128, 512], F32, name=f"pat{i}") for i in range(n_pat)]

            def rope(ps, out_slice, sc):
                cs = slice(sc * 512, (sc + 1) * 512)
                t1 = rp.tile([128, 512], F32, name="rope_t1")
                nc.vector.tensor_mul(t1[0:64, :], ps[64:128, :], sin_sb[0:64, cs])
                nc.vector.tensor_mul(t1[64:128, :], ps[0:64, :], sin_sb[64:128, cs])
                t2 = rp.tile([128, 512], F32, name="rope_t2")
                nc.vector.tensor_mul(t2[:], ps[:], cos_sb[:, cs])
                nc.vector.tensor_add(out_slice, t2[:], t1[:])

            def qkv_fillers(sc):
                """6 matmul-group closures (4 q heads, k, v) for s-chunk sc.
                The first also issues the x-tile loads."""
                cs = slice(sc * 512, (sc + 1) * 512)
                xts = []

                def load_x():
                    for e in range(ET):
                        t = xp.tile([128, 512], BF16, name=f"xts{e}")
                        nc.sync.dma_start(t[:], xT_d[e * 128:(e + 1) * 128, cs])
                        xts.append(t)

                def proj(lhs_fn, fin):
                    if not xts:
                        load_x()
                    ps = psp.tile([128, 512], F32, name="fill_ps", bufs=2)
                    for e in range(ET):
                        nc.tensor.matmul(ps[:], lhs_fn(e), xts[e][:],
                                         start=(e == 0), stop=(e == ET - 1))
                    fin(ps)

                def fq(h):
                    return lambda: proj(
                        lambda e: wq_sb[e][:, h * 128:(h + 1) * 128],
                        lambda ps: rope(ps, qT[h][:, cs], sc))

                def fk():
                    return lambda: proj(lambda e: wk_sb[e][:],
                                        lambda ps: rope(ps, kT[:, cs], sc))

                def fv():
                    def fin(ps):
                        vt = rp.tile([128, 512], BF16, name="vt_tmp")
                        nc.vector.tensor_copy(vt[:], ps[:])
                        nc.sync.dma_start_transpose(
                            v_all[:, sc * 4:(sc + 1) * 4, :], vt[:])
                    return lambda: proj(lambda e: wv_sb[e][:], fin)

                return [fq(0), fq(1), fk(), fv(), fq(2), fq(3)]

            def oproj_fillers(c, ag_out_box):
                """4 matmul-group closures for o-proj of chunk c (needs AG[c])."""
                ctxF = []

                def load_ctx():
                    for i in range(ET):
                        t = fp.tile([128, 512], BF16, name=f"ctxF{i}")
                        nc.scalar.dma_start(t[:], ag_out_box[0][i * 128:(i + 1) * 128, :])
                        ctxF.append(t)

                def grp(qs):
                    def run():
                        if not ctxF:
                            load_ctx()
                        ops = psp.tile([128, 512], F32, name="fill_ps", bufs=2)
                        for i in range(ET):
                            nc.tensor.matmul(ops[:],
                                             ctxF[i][:, qs * 128:(qs + 1) * 128],
                                             wo_sb[i][:],
                                             start=(i == 0), stop=(i == ET - 1))
                        osb = op.tile([128, 512], F32, name="osb")
                        nc.scalar.copy(osb[:], ops[:])
                        r0 = c * 512 + qs * 128
                        nc.sync.dma_start(out_d[r0:r0 + 128, :], osb[:])
                    return run

                return [grp(qs) for qs in range(4)]

            def emit_attn(c, fillers):
                """Attention for q chunk c, draining filler groups between
                dependent stages to keep TensorE dense."""
                kts = vis_kts[c]
                nfill = len(fillers)
                fi = 0

                def drain(n):
                    nonlocal fi
                    for _ in range(n):
                        if fi < nfill:
                            fillers[fi]()
                            fi += 1

                per_head = max(1, (nfill + GQ - 1) // GQ)
                ag_in = dr.tile([DL, 512], BF16, name="ag_in")
                mm2s = []
                for h in range(GQ):
                    PT = ptp.tile([128, NKT, 512], BF16, name="PT")
                    for qs in range(4):
                        qt = 4 * c + qs
                        vk = viskc[qt]
                        if not uniform:
                            for kt in kts:
                                if (kt // 4) not in vk:
                                    nc.vector.memset(
                                        PT[:, kt, qs * 128:(qs + 1) * 128], 0.0)
                        if not vk:
                            continue
                        P = pp.tile([128, S], BF16, name=f"P{qs}")
                        # contiguous runs of visible chunks
                        runs = []
                        for kc in vk:
                            if runs and runs[-1][1] == kc:
                                runs[-1][1] = kc + 1
                            else:
                                runs.append([kc, kc + 1])
                        sums = sp.tile([128, 4], F32, name="sums")
                        nsum = 0
                        for kc0, kc1 in runs:
                            # MM1 into <=2-chunk psum groups, one exp+rowsum each
                            for g0 in range(kc0, kc1, 2):
                                g1 = min(g0 + 2, kc1)
                                w = g1 - g0
                                ps = psp.tile([128, 1024], F32, name="mm1_ps",
                                              bufs=2)
                                for j, kc in enumerate(range(g0, g1)):
                                    sl = slice(j * 512, (j + 1) * 512)
                                    nc.tensor.matmul(
                                        ps[:, sl],
                                        qT[h][:, qt * 128:(qt + 1) * 128],
                                        kT[:, kc * 512:(kc + 1) * 512])
                                    code = vis[qt][kc]
                                    if code >= 2:
                                        nc.vector.tensor_add(
                                            ps[:, sl], ps[:, sl],
                                            pat_sb[code - 2][:])
                                nc.scalar.activation(
                                    P[:, g0 * 512:g1 * 512],
                                    ps[:, 0:w * 512], EXP,
                                    scale=INV_SQRT_HD,
                                    accum_out=sums[:, nsum:nsum + 1])
                                nsum += 1
                        stot = sp.tile([128, 1], F32, name="stot")
                        if nsum > 1:
                            nc.vector.tensor_reduce(
                                stot[:], sums[:, 0:nsum],
                                axis=mybir.AxisListType.X, op=ADD)
                        else:
                            nc.vector.tensor_copy(stot[:], sums[:, 0:1])
                        recip = sp.tile([128, 1], F32, name="recip")
                        nc.vector.reciprocal(recip[:], stot[:])
                        for kc0, kc1 in runs:
                            seg = slice(kc0 * 512, kc1 * 512)
                            nc.vector.tensor_scalar_mul(P[:, seg], P[:, seg],
                                                        recip[:, 0:1])
                            nc.sync.dma_start_transpose(
                                PT[:, 4 * kc0:4 * kc1, qs * 128:(qs + 1) * 128],
                                P[:, seg])

                    def mm2(h=h, PT=PT):
                        ctx_ps = psp.tile([128, 512], F32, name="ctx_ps", bufs=2)
                        for j, kt in enumerate(kts):
                            nc.tensor.matmul(ctx_ps[:], v_all[:, kt, :],
                                             PT[:, kt, :], start=(j == 0),
                                             stop=(j == len(kts) - 1))
                        ctx_sb = cp.tile([128, 512], BF16, name=f"ctx{h}")
                        nc.vector.tensor_copy(ctx_sb[:], ctx_ps[:])
                        nc.sync.dma_start(ag_in[h * 128:(h + 1) * 128, :],
                                          ctx_sb[:])
                    mm2s.append(mm2)
                    # fillers run while this head's exp/scale/transpose drain;
                    # then the PREVIOUS head's MM2 (its P^T is ready by now)
                    drain(per_head)
                    if h >= 1:
                        mm2s.pop(0)()
                drain(nfill)
                while mm2s:
                    mm2s.pop(0)()

                ag_out = dr.tile([H, 512], BF16, name="ag_out")
                nc.gpsimd.collective_compute(
                    "AllGather", mybir.AluOpType.bypass,
                    replica_groups=[[0, 1, 2, 3], [4, 5, 6, 7]],
                    ins=[ag_in[:].opt()], outs=[ag_out[:].opt()])
                return ag_out

            # ---- pipelined emission ----
            boxes = {c: [None] for c in range(NSC)}
            for f in qkv_fillers(0):
                f()
            for i in range(n_pat):
                nc.scalar.dma_start(pat_sb[i][:], pats_d[i])
            for e in range(ET):
                nc.scalar.dma_start(wo_sb[e][:], woT_d[e * 128:(e + 1) * 128, :])

            boxes[0][0] = emit_attn(0, qkv_fillers(1))
            boxes[1][0] = emit_attn(1, qkv_fillers(2) + oproj_fillers(0, boxes[0]))
            boxes[2][0] = emit_attn(2, qkv_fillers(3) + oproj_fillers(1, boxes[1]))
            boxes[3][0] = emit_attn(3, oproj_fillers(2, boxes[2]))
            for f in oproj_fillers(3, boxes[3]):
                f()

    nc.compile()
    return nc


def kernel(hidden_states, wq, wk, wv, wo, attention_mask, position_ids):
    hidden_states = np.asarray(hidden_states, dtype=np.float32)
    wq = np.asarray(wq, dtype=np.float32)
    wk = np.asarray(wk, dtype=np.float32)
    wv = np.asarray(wv, dtype=np.float32)
    wo = np.asarray(wo, dtype=np.float32)
    pos = np.asarray(position_ids)

    vis, pats, n_pat = _classify_mask(attention_mask)
    key = (tuple(tuple(r) for r in vis), n_pat)
    if key not in _cache:
        _cache[key] = _build(vis, n_pat)
    nc = _cache[key]

    # RoPE tables per batch: cosT/sinT [HD, S]; sinT sign-folded (-sin for d<64)
    inv_freq = 1.0 / (ROPE_THETA ** (np.arange(0, HD, 2, dtype=np.float32) / HD))
    cosT = np.empty((B, HD, S), np.float32)
    sinT = np.empty((B, HD, S), np.float32)
    for b in range(B):
        freqs = pos[b].astype(np.float32)[None, :] * inv_freq[:, None]  # [64, S]
        cosT[b] = np.concatenate([np.cos(freqs)] * 2, axis=0)
        sn = np.sin(freqs)
        sinT[b] = np.concatenate([-sn, sn], axis=0)

    bf = ml_dtypes.bfloat16
    xT = [np.ascontiguousarray(hidden_states[b].T).astype(bf) for b in range(B)]
    in_maps = []
    for c in range(8):
        b, g = c // 4, c % 4
        in_maps.append({
            "xT": xT[b],
            "wqT": np.ascontiguousarray(wq[g * DL:(g + 1) * DL, :].T).astype(bf),
            "wkT": np.ascontiguousarray(wk[g * HD:(g + 1) * HD, :].T).astype(bf),
            "wvT": np.ascontiguousarray(wv[g * HD:(g + 1) * HD, :].T).astype(bf),
            "woT": np.ascontiguousarray(wo[g * DL:(g + 1) * DL, :].T).astype(bf),
            "cosT": cosT[b],
            "sinT": sinT[b],
            "pats": pats[b],
        })

    res = run_bass_kernel_spmd(nc, in_maps, core_ids=list(range(8))).results
    out = np.empty((B, S, H), np.float32)
    for c in range(8):
        b, g = c // 4, c % 4
        out[b, :, g * DL:(g + 1) * DL] = res[c]["out"]
    return out


# revision 7
# speedup vs baseline: 1.1089x; 1.0110x over previous
"""GQA attention layer (B=2, S=2048, H=2048, 16 q heads / 4 kv heads, RoPE, causal
mask) on 8 TRN2 NeuronCores.

Sharding: core c = (b, g) with b = c // 4 (batch), g = c % 4 (kv-head group).
Each core computes q-heads 4g..4g+3 and kv-head g for batch b:
  - QKV projections from x^T (bf16 matmuls, f32 PSUM accumulate)
  - RoPE on q/k (f32, host-precomputed cos/sin tables)
  - MM1 scores [q, k] -> mask -> fused exp+rowsum on ScalarE -> P (bf16)
  - P scaled by 1/rowsum, DMA-transposed -> P^T, MM2 -> ctx^T [d, q]
  - AllGather ctx^T across the 4 cores of the batch (groups [[0..3],[4..7]])
  - o-proj: each core computes its 512-column block of the output for all S.
Host reassembles [B, S, H] from the 8 [S, 512] column blocks.

The additive attention mask is handled generally: each 128x512 score tile is
classified at build time (from the actual mask input) as fully-masked (matmul
skipped), zero (no mask op), or mixed (a per-core mask pattern tile is added
pre-exp). For the causal mask this skips the upper triangle (~half the
attention FLOPs) and needs only 4 distinct pattern tiles.

Scheduling: the attention chain (MM1 -> exp on ScalarE -> scale -> DMA
transpose -> MM2) is latency- and ScalarE-bound, which starves TensorE and
triggers HAM re-throttling to half clock. So independent matmul work (QKV
projection of the next chunk, o-proj of the previous chunk) is fed into the
emission stream at single-matmul granularity between the dependent attention
matmuls, keeping TensorE dense. ScalarE runs exp exclusively (copies live on
VectorE) to avoid activation-table thrash.
"""
import math
import os
import sys

for _p in ("/opt/trn_rl_repo",):
    if _p not in sys.path and os.path.isdir(_p):
        sys.path.insert(0, _p)

import ml_dtypes
import numpy as np

from concourse import bacc, mybir, tile
from concourse.bass_utils import run_bass_kernel_spmd

BF16 = mybir.dt.bfloat16
F32 = mybir.dt.float32
EXP = mybir.ActivationFunctionType.Exp
ADD = mybir.AluOpType.add

B, S, H = 2, 2048, 2048
NH, NKV, HD = 16, 4, 128
GQ = NH // NKV            # q heads per core (4)
DL = GQ * HD              # local q width (512)
ET = H // 128             # e-tiles (16)
NSC = S // 512            # 512-wide s/k chunks (4)
NQT = S // 128            # 128-row q tiles (16)
NKT = S // 128            # 128-row k tiles (16)
ROPE_THETA = 10000.0
INV_SQRT_HD = 1.0 / math.sqrt(HD)
SQRT_HD = math.sqrt(HD)

SKIP, FREE = 0, 1         # vis codes; >=2 means pattern index (code - 2)

_cache = {}


def _classify_mask(mask):
    """mask: [B, 1, S, S] f32 additive. Returns (vis, pats) where
    vis[qt][kc] in {SKIP, FREE, idx+2} and pats[b] is [n_pat, 128, 512] f32
    (clipped, pre-multiplied by sqrt(HD))."""
    m = np.asarray(mask, dtype=np.float32).reshape(B, S, S)
    vis = [[FREE] * NSC for _ in range(NQT)]
    pat_ids = {}
    pats = [[] for _ in range(B)]
    for qt in range(NQT):
        for kc in range(NSC):
            blk = m[:, qt * 128:(qt + 1) * 128, kc * 512:(kc + 1) * 512]
            if np.all(blk <= -1e8):
                vis[qt][kc] = SKIP
            elif np.all(blk == 0.0):
                vis[qt][kc] = FREE
            else:
                clipped = np.maximum(blk, -90.0) * SQRT_HD
                key = clipped.tobytes()
                if key not in pat_ids:
                    pat_ids[key] = len(pats[0])
                    for b in range(B):
                        pats[b].append(clipped[b])
                vis[qt][kc] = pat_ids[key] + 2
    n_pat = len(pats[0])
    if n_pat == 0:
        pats_np = [np.zeros((1, 128, 512), np.float32) for _ in range(B)]
        n_pat = 1
    else:
        pats_np = [np.stack(p) for p in pats]
    return vis, pats_np, n_pat


class Feeder:
    """Round-robin pull of single-matmul filler steps from generators."""

    def __init__(self):
        self.gens = []

    def add(self, gen):
        self.gens.append(gen)

    def pull(self, n):
        while n > 0 and self.gens:
            try:
                next(self.gens[0])
                n -= 1
            except StopIteration:
                self.gens.pop(0)

    def drain(self):
        while self.gens:
            self.pull(1 << 30)


def _build(vis, n_pat):
    nc = bacc.Bacc(None, target_bir_lowering=False, num_devices=8)

    xT_d = nc.dram_tensor("xT", [H, S], BF16, kind="ExternalInput")
    wqT_d = nc.dram_tensor("wqT", [H, DL], BF16, kind="ExternalInput")
    wkT_d = nc.dram_tensor("wkT", [H, HD], BF16, kind="ExternalInput")
    wvT_d = nc.dram_tensor("wvT", [H, HD], BF16, kind="ExternalInput")
    woT_d = nc.dram_tensor("woT", [H, DL], BF16, kind="ExternalInput")
    cosT_d = nc.dram_tensor("cosT", [HD, S], F32, kind="ExternalInput")
    sinT_d = nc.dram_tensor("sinT", [HD, S], F32, kind="ExternalInput")
    pats_d = nc.dram_tensor("pats", [n_pat, 128, 512], F32, kind="ExternalInput")
    out_d = nc.dram_tensor("out", [S, DL], F32, kind="ExternalOutput")

    viskc = [[kc for kc in range(NSC) if vis[qt][kc] != SKIP] for qt in range(NQT)]
    vis_kts = []
    for c in range(NSC):
        kts = sorted({kt for qs in range(4) for kc in viskc[4 * c + qs]
                      for kt in range(4 * kc, 4 * kc + 4)})
        vis_kts.append(kts)
    uniform = all(
        all(viskc[4 * c + qs] == viskc[4 * c] for qs in range(4)) for c in range(NSC)
    )

    with tile.TileContext(nc) as tc:
        with (
            tc.tile_pool(name="wp", bufs=1) as wp,
            tc.tile_pool(name="xp", bufs=2) as xp,
            tc.tile_pool(name="qk", bufs=1) as qk,
            tc.tile_pool(name="rp", bufs=2) as rp,
            tc.tile_pool(name="pp", bufs=1) as pp,
            tc.tile_pool(name="ptp", bufs=2) as ptp,
            tc.tile_pool(name="cp", bufs=2) as cp,
            tc.tile_pool(name="fp", bufs=1) as fp,
            tc.tile_pool(name="op", bufs=2) as op,
            tc.tile_pool(name="sp", bufs=6) as sp,
            tc.tile_pool(name="ps", bufs=1, space="PSUM") as psp,
            tc.tile_pool(name="dr", bufs=2, space="DRAM") as dr,
        ):
            # ---- weights / tables for phase 1; wq/x interleaved and split
            # across both HWDGE trigger engines so the first projection
            # group's operands arrive first ----
            wq_sb = [wp.tile([128, DL], BF16, name=f"wq{e}") for e in range(ET)]
            wk_sb = [wp.tile([128, HD], BF16, name=f"wk{e}") for e in range(ET)]
            wv_sb = [wp.tile([128, HD], BF16, name=f"wv{e}") for e in range(ET)]
            for e in range(ET):
                r = slice(e * 128, (e + 1) * 128)
                nc.sync.dma_start(wq_sb[e][:], wqT_d[r, :])
            for e in range(ET):
                r = slice(e * 128, (e + 1) * 128)
                nc.scalar.dma_start(wk_sb[e][:], wkT_d[r, :])
                nc.scalar.dma_start(wv_sb[e][:], wvT_d[r, :])
            cos_sb = wp.tile([HD, S], F32, name="cos_sb")
            sin_sb = wp.tile([HD, S], F32, name="sin_sb")
            nc.scalar.dma_start(cos_sb[:], cosT_d[:])
            nc.scalar.dma_start(sin_sb[:], sinT_d[:])

            # ---- persistent activations ----
            qT = [qk.tile([HD, S], BF16, name=f"qT{h}") for h in range(GQ)]
            kT = qk.tile([HD, S], BF16, name="kT")
            v_all = qk.tile([128, NKT, HD], BF16, name="v_all")
            wo_sb = [wp.tile([128, DL], BF16, name=f"wo{e}") for e in range(ET)]
            pat_sb = [wp.tile([128, 512], F32, name=f"pat{i}") for i in range(n_pat)]

            def rope(ps, out_slice, sc):
                cs = slice(sc * 512, (sc + 1) * 512)
                t1 = rp.tile([128, 512], F32, name="rope_t1")
                nc.vector.tensor_mul(t1[0:64, :], ps[64:128, :], sin_sb[0:64, cs])
                nc.vector.tensor_mul(t1[64:128, :], ps[0:64, :], sin_sb[64:128, cs])
                t2 = rp.tile([128, 512], F32, name="rope_t2")
                nc.vector.tensor_mul(t2[:], ps[:], cos_sb[:, cs])
                nc.vector.tensor_add(out_slice, t2[:], t1[:])

            def qkv_gen(sc):
                """Yields once per matmul; 6 groups (4 q heads, k, v)."""
                cs = slice(sc * 512, (sc + 1) * 512)
                xts = []
                for e in range(ET):
                    t = xp.tile([128, 512], BF16, name=f"xts{e}")
                    nc.sync.dma_start(t[:], xT_d[e * 128:(e + 1) * 128, cs])
                    xts.append(t)

                def fin_q(h):
                    return lambda ps: rope(ps, qT[h][:, cs], sc)

                def fin_k(ps):
                    rope(ps, kT[:, cs], sc)

                def fin_v(ps):
                    vt = rp.tile([128, 512], BF16, name="vt_tmp")
                    nc.vector.tensor_copy(vt[:], ps[:])
                    nc.sync.dma_start_transpose(
                        v_all[:, sc * 4:(sc + 1) * 4, :], vt[:])

                blocks = [
                    (lambda e, h=h: wq_sb[e][:, h * 128:(h + 1) * 128], fin_q(h))
                    for h in range(2)
                ] + [
                    (lambda e: wk_sb[e][:], fin_k),
                    (lambda e: wv_sb[e][:], fin_v),
                ] + [
                    (lambda e, h=h: wq_sb[e][:, h * 128:(h + 1) * 128], fin_q(h))
                    for h in range(2, GQ)
                ]
                for lhs_fn, fin in blocks:
                    ps = psp.tile([128, 512], F32, name="fill_ps", bufs=2)
                    for e in range(ET):
                        nc.tensor.matmul(ps[:], lhs_fn(e), xts[e][:],
                                         start=(e == 0), stop=(e == ET - 1))
                        yield
                    fin(ps)

            def oproj_gen(c, ag_out_box):
                """Yields once per matmul; 4 groups of 16 (o-proj chunk c)."""
                ctxF = []
                for i in range(ET):
                    t = fp.tile([128, 512], BF16, name=f"ctxF{i}")
                    nc.sync.dma_start(t[:], ag_out_box[0][i * 128:(i + 1) * 128, :])
                    ctxF.append(t)
                for qs in range(4):
                    ops = psp.tile([128, 512], F32, name="fill_ps", bufs=2)
                    for i in range(ET):
                        nc.tensor.matmul(ops[:],
                                         ctxF[i][:, qs * 128:(qs + 1) * 128],
                                         wo_sb[i][:],
                                         start=(i == 0), stop=(i == ET - 1))
                        yield
                    osb = op.tile([128, 512], F32, name="osb")
                    nc.vector.tensor_copy(osb[:], ops[:])
                    r0 = c * 512 + qs * 128
                    nc.sync.dma_start(out_d[r0:r0 + 128, :], osb[:])

            def emit_attn(c, feeder, n_fill):
                """Attention for q chunk c; pulls filler matmuls between the
                dependent stages to keep TensorE dense."""
                kts = vis_kts[c]
                # filler matmuls available per MM1/MM2 matmul emitted
                n_attn_mm = sum(len(viskc[4 * c + qs]) for qs in range(4)) * GQ * 2
                ratio = n_fill / max(1, n_attn_mm)
                acc = 0.0

                def tick(k=1):
                    nonlocal acc
                    acc += ratio * k
                    n = int(acc)
                    if n:
                        feeder.pull(n)
                        acc -= n

                ag_in = dr.tile([DL, 512], BF16, name="ag_in")
                mm2s = []
                for h in range(GQ):
                    PT = ptp.tile([128, NKT, 512], BF16, name="PT")
                    for qs in range(4):
                        qt = 4 * c + qs
                        vk = viskc[qt]
                        if not uniform:
                            for kt in kts:
                                if (kt // 4) not in vk:
                                    nc.vector.memset(
                                        PT[:, kt, qs * 128:(qs + 1) * 128], 0.0)
                        if not vk:
                            continue
                        P = pp.tile([128, S], BF16, name=f"P{qs}")
                        runs = []
                        for kc in vk:
                            if runs and runs[-1][1] == kc:
                                runs[-1][1] = kc + 1
                            else:
                                runs.append([kc, kc + 1])
                        sums = sp.tile([128, 4], F32, name="sums")
                        nsum = 0
                        for kc0, kc1 in runs:
                            for g0 in range(kc0, kc1, 2):
                                g1 = min(g0 + 2, kc1)
                                w = g1 - g0
                                ps = psp.tile([128, 1024], F32, name="mm1_ps",
                                              bufs=2)
                                for j, kc in enumerate(range(g0, g1)):
                                    sl = slice(j * 512, (j + 1) * 512)
                                    nc.tensor.matmul(
                                        ps[:, sl],
                                        qT[h][:, qt * 128:(qt + 1) * 128],
                                        kT[:, kc * 512:(kc + 1) * 512])
                                    tick()
                                    code = vis[qt][kc]
                                    if code >= 2:
                                        nc.vector.tensor_add(
                                            ps[:, sl], ps[:, sl],
                                            pat_sb[code - 2][:])
                                nc.scalar.activation(
                                    P[:, g0 * 512:g1 * 512],
                                    ps[:, 0:w * 512], EXP,
                                    scale=INV_SQRT_HD,
                                    accum_out=sums[:, nsum:nsum + 1])
                                nsum += 1
                        stot = sp.tile([128, 1], F32, name="stot")
                        if nsum > 1:
                            nc.vector.tensor_reduce(
                                stot[:], sums[:, 0:nsum],
                                axis=mybir.AxisListType.X, op=ADD)
                        else:
                            nc.vector.tensor_copy(stot[:], sums[:, 0:1])
                        recip = sp.tile([128, 1], F32, name="recip")
                        nc.vector.reciprocal(recip[:], stot[:])
                        for kc0, kc1 in runs:
                            seg = slice(kc0 * 512, kc1 * 512)
                            nc.vector.tensor_scalar_mul(P[:, seg], P[:, seg],
                                                        recip[:, 0:1])
                            nc.sync.dma_start_transpose(
                                PT[:, 4 * kc0:4 * kc1, qs * 128:(qs + 1) * 128],
                                P[:, seg])

                    def mm2(h=h, PT=PT):
                        ctx_ps = psp.tile([128, 512], F32, name="ctx_ps", bufs=2)
                        for j, kt in enumerate(kts):
                            nc.tensor.matmul(ctx_ps[:], v_all[:, kt, :],
                                             PT[:, kt, :], start=(j == 0),
                                             stop=(j == len(kts) - 1))
                            tick()
                        ctx_sb = cp.tile([128, 512], BF16, name=f"ctx{h}")
                        nc.vector.tensor_copy(ctx_sb[:], ctx_ps[:])
                        nc.sync.dma_start(ag_in[h * 128:(h + 1) * 128, :],
                                          ctx_sb[:])
                    mm2s.append(mm2)
                    if h >= 1:
                        mm2s.pop(0)()
                while mm2s:
                    mm2s.pop(0)()

                ag_out = dr.tile([H, 512], BF16, name="ag_out")
                nc.gpsimd.collective_compute(
                    "AllGather", mybir.AluOpType.bypass,
                    replica_groups=[[0, 1, 2, 3], [4, 5, 6, 7]],
                    ins=[ag_in[:].opt()], outs=[ag_out[:].opt()])
                return ag_out

            # ---- pipelined emission ----
            boxes = {c: [None] for c in range(NSC)}
            f0 = Feeder()
            f0.add(qkv_gen(0))
            f0.drain()
            for i in range(n_pat):
                nc.scalar.dma_start(pat_sb[i][:], pats_d[i])
            for e in range(ET):
                nc.scalar.dma_start(wo_sb[e][:], woT_d[e * 128:(e + 1) * 128, :])

            fd = Feeder()
            fd.add(qkv_gen(1))
            boxes[0][0] = emit_attn(0, fd, 96)
            fd.drain()
            fd.add(qkv_gen(2))
            fd.add(oproj_gen(0, boxes[0]))
            boxes[1][0] = emit_attn(1, fd, 160)
            fd.drain()
            fd.add(qkv_gen(3))
            fd.add(oproj_gen(1, boxes[1]))
            boxes[2][0] = emit_attn(2, fd, 160)
            fd.drain()
            fd.add(oproj_gen(2, boxes[2]))
            boxes[3][0] = emit_attn(3, fd, 64)
            fd.drain()
            fd.add(oproj_gen(3, boxes[3]))
            fd.drain()

    nc.compile()
    return nc


def kernel(hidden_states, wq, wk, wv, wo, attention_mask, position_ids):
    hidden_states = np.asarray(hidden_states, dtype=np.float32)
    wq = np.asarray(wq, dtype=np.float32)
    wk = np.asarray(wk, dtype=np.float32)
    wv = np.asarray(wv, dtype=np.float32)
    wo = np.asarray(wo, dtype=np.float32)
    pos = np.asarray(position_ids)

    vis, pats, n_pat = _classify_mask(attention_mask)
    key = (tuple(tuple(r) for r in vis), n_pat)
    if key not in _cache:
        _cache[key] = _build(vis, n_pat)
    nc = _cache[key]

    # RoPE tables per batch: cosT/sinT [HD, S]; sinT sign-folded (-sin for d<64)
    inv_freq = 1.0 / (ROPE_THETA ** (np.arange(0, HD, 2, dtype=np.float32) / HD))
    cosT = np.empty((B, HD, S), np.float32)
    sinT = np.empty((B, HD, S), np.float32)
    for b in range(B):
        freqs = pos[b].astype(np.float32)[None, :] * inv_freq[:, None]  # [64, S]
        cosT[b] = np.concatenate([np.cos(freqs)] * 2, axis=0)
        sn = np.sin(freqs)
        sinT[b] = np.concatenate([-sn, sn], axis=0)

    bf = ml_dtypes.bfloat16
    xT = [np.ascontiguousarray(hidden_states[b].T).astype(bf) for b in range(B)]
    in_maps = []
    for c in range(8):
        b, g = c // 4, c % 4
        in_maps.append({
            "xT": xT[b],
            "wqT": np.ascontiguousarray(wq[g * DL:(g + 1) * DL, :].T).astype(bf),
            "wkT": np.ascontiguousarray(wk[g * HD:(g + 1) * HD, :].T).astype(bf),
            "wvT": np.ascontiguousarray(wv[g * HD:(g + 1) * HD, :].T).astype(bf),
            "woT": np.ascontiguousarray(wo[g * DL:(g + 1) * DL, :].T).astype(bf),
            "cosT": cosT[b],
            "sinT": sinT[b],
            "pats": pats[b],
        })

    res = run_bass_kernel_spmd(nc, in_maps, core_ids=list(range(8))).results
    out = np.empty((B, S, H), np.float32)
    for c in range(8):
        b, g = c // 4, c % 4
        out[b, :, g * DL:(g + 1) * DL] = res[c]["out"]
    return out


# revision 16
# speedup vs baseline: 1.1170x; 1.0073x over previous
"""GQA attention layer (B=2, S=2048, H=2048, 16 q heads / 4 kv heads, RoPE, causal
mask) on 8 TRN2 NeuronCores.

Sharding: core c = (b, g) with b = c // 4 (batch), g = c % 4 (kv-head group).
Each core computes q-heads 4g..4g+3 and kv-head g for batch b:
  - QKV projections from x^T (bf16 matmuls, f32 PSUM accumulate)
  - RoPE on q/k (f32, host-precomputed cos/sin tables)
  - MM1 scores [q, k] -> mask -> fused exp+rowsum on ScalarE -> P (bf16)
  - P scaled by 1/rowsum, DMA-transposed -> P^T, MM2 -> ctx^T [d, q]
  - AllGather ctx^T across the 4 cores of the batch (groups [[0..3],[4..7]])
  - o-proj: each core computes its 512-column block of the output for all S.
Host reassembles [B, S, H] from the 8 [S, 512] column blocks.

The additive attention mask is handled generally: each 128x512 score tile is
classified at build time (from the actual mask input) as fully-masked (matmul
skipped), zero (no mask op), or mixed (a per-core mask pattern tile is added
pre-exp). For the causal mask this skips the upper triangle (~half the
attention FLOPs) and needs only 4 distinct pattern tiles.

Scheduling: the attention chain (MM1 -> exp on ScalarE -> scale -> DMA
transpose -> MM2) is latency- and ScalarE-bound, which starves TensorE and
triggers HAM re-throttling to half clock. So independent matmul work (QKV
projection of the next chunk, o-proj of the previous chunk) is fed into the
emission stream at single-matmul granularity between the dependent attention
matmuls, keeping TensorE dense. ScalarE runs exp exclusively (copies live on
VectorE) to avoid activation-table thrash.
"""
import math
import os
import sys

for _p in ("/opt/trn_rl_repo",):
    if _p not in sys.path and os.path.isdir(_p):
        sys.path.insert(0, _p)

import ml_dtypes
import numpy as np

from concourse import bacc, mybir, tile
from concourse.bass_utils import run_bass_kernel_spmd

BF16 = mybir.dt.bfloat16
F32 = mybir.dt.float32
EXP = mybir.ActivationFunctionType.Exp
ADD = mybir.AluOpType.add

B, S, H = 2, 2048, 2048
NH, NKV, HD = 16, 4, 128
GQ = NH // NKV            # q heads per core (4)
DL = GQ * HD              # local q width (512)
ET = H // 128             # e-tiles (16)
NSC = S // 512            # 512-wide s/k chunks (4)
NQT = S // 128            # 128-row q tiles (16)
NKT = S // 128            # 128-row k tiles (16)
ROPE_THETA = 10000.0
INV_SQRT_HD = 1.0 / math.sqrt(HD)
SQRT_HD = math.sqrt(HD)

SKIP, FREE = 0, 1         # vis codes; >=2 means pattern index (code - 2)

_cache = {}


def _classify_mask(mask):
    """mask: [B, 1, S, S] f32 additive. Returns (vis, pats) where
    vis[qt][kc] in {SKIP, FREE, idx+2} and pats[b] is [n_pat, 128, 512] f32
    (clipped, pre-multiplied by sqrt(HD))."""
    m = np.asarray(mask, dtype=np.float32).reshape(B, S, S)
    vis = [[FREE] * NSC for _ in range(NQT)]
    pat_ids = {}
    pats = [[] for _ in range(B)]
    for qt in range(NQT):
        for kc in range(NSC):
            blk = m[:, qt * 128:(qt + 1) * 128, kc * 512:(kc + 1) * 512]
            if np.all(blk <= -1e8):
                vis[qt][kc] = SKIP
            elif np.all(blk == 0.0):
                vis[qt][kc] = FREE
            else:
                clipped = np.maximum(blk, -90.0) * SQRT_HD
                key = clipped.tobytes()
                if key not in pat_ids:
                    pat_ids[key] = len(pats[0])
                    for b in range(B):
                        pats[b].append(clipped[b])
                vis[qt][kc] = pat_ids[key] + 2
    n_pat = len(pats[0])
    if n_pat == 0:
        pats_np = [np.zeros((1, 128, 512), np.float32) for _ in range(B)]
        n_pat = 1
    else:
        pats_np = [np.stack(p) for p in pats]
    return vis, pats_np, n_pat


class Feeder:
    """Round-robin pull of single-matmul filler steps from generators."""

    def __init__(self):
        self.gens = []

    def add(self, gen):
        self.gens.append(gen)

    def pull(self, n):
        while n > 0 and self.gens:
            try:
                next(self.gens[0])
                n -= 1
            except StopIteration:
                self.gens.pop(0)

    def drain(self):
        while self.gens:
            self.pull(1 << 30)


def _build(vis, n_pat):
    nc = bacc.Bacc(None, target_bir_lowering=False, num_devices=8)

    xT_d = nc.dram_tensor("xT", [H, S], BF16, kind="ExternalInput")
    wqT_d = nc.dram_tensor("wqT", [H, DL], BF16, kind="ExternalInput")
    wkT_d = nc.dram_tensor("wkT", [H, HD], BF16, kind="ExternalInput")
    wvT_d = nc.dram_tensor("wvT", [H, HD], BF16, kind="ExternalInput")
    woT_d = nc.dram_tensor("woT", [H, DL], BF16, kind="ExternalInput")
    cosT_d = nc.dram_tensor("cosT", [HD, S], F32, kind="ExternalInput")
    sinT_d = nc.dram_tensor("sinT", [HD, S], F32, kind="ExternalInput")
    pats_d = nc.dram_tensor("pats", [n_pat, 128, 512], F32, kind="ExternalInput")
    out_d = nc.dram_tensor("out", [S, DL], F32, kind="ExternalOutput")

    viskc = [[kc for kc in range(NSC) if vis[qt][kc] != SKIP] for qt in range(NQT)]
    vis_kts = []
    for c in range(NSC):
        kts = sorted({kt for qs in range(4) for kc in viskc[4 * c + qs]
                      for kt in range(4 * kc, 4 * kc + 4)})
        vis_kts.append(kts)
    uniform = all(
        all(viskc[4 * c + qs] == viskc[4 * c] for qs in range(4)) for c in range(NSC)
    )

    with tile.TileContext(nc) as tc:
        with (
            tc.tile_pool(name="wp", bufs=1) as wp,
            tc.tile_pool(name="xp", bufs=2) as xp,
            tc.tile_pool(name="qk", bufs=1) as qk,
            tc.tile_pool(name="rp", bufs=2) as rp,
            tc.tile_pool(name="pp", bufs=1) as pp,
            tc.tile_pool(name="ptp", bufs=2) as ptp,
            tc.tile_pool(name="cp", bufs=2) as cp,
            tc.tile_pool(name="fp", bufs=1) as fp,
            tc.tile_pool(name="op", bufs=2) as op,
            tc.tile_pool(name="sp", bufs=6) as sp,
            tc.tile_pool(name="ps", bufs=1, space="PSUM") as psp,
            tc.tile_pool(name="dr", bufs=2, space="DRAM") as dr,
        ):
            # ---- weights / tables for phase 1; wq/x interleaved and split
            # across both HWDGE trigger engines so the first projection
            # group's operands arrive first ----
            wq_sb = [wp.tile([128, DL], BF16, name=f"wq{e}") for e in range(ET)]
            wk_sb = [wp.tile([128, HD], BF16, name=f"wk{e}") for e in range(ET)]
            wv_sb = [wp.tile([128, HD], BF16, name=f"wv{e}") for e in range(ET)]
            for e in range(ET):
                r = slice(e * 128, (e + 1) * 128)
                nc.sync.dma_start(wq_sb[e][:], wqT_d[r, :])
            for e in range(ET):
                r = slice(e * 128, (e + 1) * 128)
                nc.scalar.dma_start(wk_sb[e][:], wkT_d[r, :])
                nc.scalar.dma_start(wv_sb[e][:], wvT_d[r, :])
            cos_sb = wp.tile([HD, S], F32, name="cos_sb")
            sin_sb = wp.tile([HD, S], F32, name="sin_sb")
            nc.scalar.dma_start(cos_sb[:], cosT_d[:])
            nc.scalar.dma_start(sin_sb[:], sinT_d[:])

            # ---- persistent activations ----
            qT = [qk.tile([HD, S], BF16, name=f"qT{h}") for h in range(GQ)]
            kT = qk.tile([HD, S], BF16, name="kT")
            v_all = qk.tile([128, NKT, HD], BF16, name="v_all")
            wo_sb = [wp.tile([128, DL], BF16, name=f"wo{e}") for e in range(ET)]
            pat_sb = [wp.tile([128, 512], F32, name=f"pat{i}") for i in range(n_pat)]

            def rope(ps, out_slice, sc):
                cs = slice(sc * 512, (sc + 1) * 512)
                t1 = rp.tile([128, 512], F32, name="rope_t1")
                nc.vector.tensor_mul(t1[0:64, :], ps[64:128, :], sin_sb[0:64, cs])
                nc.vector.tensor_mul(t1[64:128, :], ps[0:64, :], sin_sb[64:128, cs])
                t2 = rp.tile([128, 512], F32, name="rope_t2")
                nc.vector.tensor_mul(t2[:], ps[:], cos_sb[:, cs])
                nc.vector.tensor_add(out_slice, t2[:], t1[:])

            def qkv_gen(sc):
                """Yields once per matmul; 6 groups (4 q heads, k, v)."""
                cs = slice(sc * 512, (sc + 1) * 512)
                xts = []
                for e in range(ET):
                    t = xp.tile([128, 512], BF16, name=f"xts{e}")
                    nc.sync.dma_start(t[:], xT_d[e * 128:(e + 1) * 128, cs])
                    xts.append(t)

                def fin_q(h):
                    return lambda ps: rope(ps, qT[h][:, cs], sc)

                def fin_k(ps):
                    rope(ps, kT[:, cs], sc)

                def fin_v(ps):
                    vt = rp.tile([128, 512], BF16, name="vt_tmp")
                    nc.vector.tensor_copy(vt[:], ps[:])
                    nc.sync.dma_start_transpose(
                        v_all[:, sc * 4:(sc + 1) * 4, :], vt[:])

                blocks = [
                    (lambda e, h=h: wq_sb[e][:, h * 128:(h + 1) * 128], fin_q(h))
                    for h in range(2)
                ] + [
                    (lambda e: wk_sb[e][:], fin_k),
                    (lambda e: wv_sb[e][:], fin_v),
                ] + [
                    (lambda e, h=h: wq_sb[e][:, h * 128:(h + 1) * 128], fin_q(h))
                    for h in range(2, GQ)
                ]
                for lhs_fn, fin in blocks:
                    ps = psp.tile([128, 512], F32, name="fill_ps", bufs=2)
                    for e in range(ET):
                        nc.tensor.matmul(ps[:], lhs_fn(e), xts[e][:],
                                         start=(e == 0), stop=(e == ET - 1))
                        yield
                    fin(ps)

            def oproj_start(c, ag_outs):
                """Eagerly issue the gathered-ctx loads for o-proj chunk c.
                ag_outs: list of (ag_out, itiles) covering all 16 i-tiles."""
                ctxF = [None] * ET
                for ag_out, itiles in ag_outs:
                    for j, i in enumerate(itiles):
                        t = fp.tile([128, 512], BF16, name=f"ctxF{i}")
                        nc.scalar.dma_start(t[:], ag_out[j * 128:(j + 1) * 128, :])
                        ctxF[i] = t
                return ctxF

            def oproj_gen(c, ctxF, iorder=None):
                """Yields once per matmul; 4 groups of 16 (o-proj chunk c)."""
                iorder = iorder or list(range(ET))
                for qs in range(4):
                    ops = psp.tile([128, 512], F32, name="fill_ps", bufs=2)
                    for j, i in enumerate(iorder):
                        nc.tensor.matmul(ops[:],
                                         ctxF[i][:, qs * 128:(qs + 1) * 128],
                                         wo_sb[i][:],
                                         start=(j == 0), stop=(j == ET - 1))
                        yield
                    osb = op.tile([128, 512], F32, name="osb")
                    nc.vector.tensor_copy(osb[:], ops[:])
                    r0 = c * 512 + qs * 128
                    nc.sync.dma_start(out_d[r0:r0 + 128, :], osb[:])

            def emit_attn(c, feeder, n_fill, split_ag=False):
                """Attention for q chunk c; pulls filler matmuls between the
                dependent stages to keep TensorE dense. Returns a list of
                (ag_out, itiles): the AllGather output dram tiles and the
                global i-tile (head*128 row-block) order inside each."""
                kts = vis_kts[c]
                # filler matmuls available per MM1/MM2 matmul emitted
                n_attn_mm = sum(len(viskc[4 * c + qs]) for qs in range(4)) * GQ * 2
                ratio = n_fill / max(1, n_attn_mm)
                acc = 0.0

                def tick(k=1):
                    nonlocal acc
                    acc += ratio * k
                    n = int(acc)
                    if n:
                        feeder.pull(n)
                        acc -= n

                if split_ag:
                    ag_ins = [dr.tile([2 * HD, 512], BF16, name="ag_in_a"),
                              dr.tile([2 * HD, 512], BF16, name="ag_in_b")]
                else:
                    ag_ins = [dr.tile([DL, 512], BF16, name="ag_in")]
                ags = []

                def issue_ag(part):
                    ag_in = ag_ins[part]
                    nr = ag_in.shape[0]
                    ag_out = dr.tile([4 * nr, 512], BF16,
                                     name=f"ag_out{'ab'[part] if split_ag else ''}")
                    nc.gpsimd.collective_compute(
                        "AllGather", mybir.AluOpType.bypass,
                        replica_groups=[[0, 1, 2, 3], [4, 5, 6, 7]],
                        ins=[ag_in[:].opt()], outs=[ag_out[:].opt()])
                    nh_part = nr // HD
                    itiles = [4 * r + part * nh_part + h
                              for r in range(4) for h in range(nh_part)]
                    ags.append((ag_out, itiles))

                done_mm2 = [0]
                mm2s = []
                for h in range(GQ):
                    PT = ptp.tile([128, NKT, 512], BF16, name="PT")
                    for qs in range(4):
                        qt = 4 * c + qs
                        vk = viskc[qt]
                        if not uniform:
                            for kt in kts:
                                if (kt // 4) not in vk:
                                    nc.vector.memset(
                                        PT[:, kt, qs * 128:(qs + 1) * 128], 0.0)
                        if not vk:
                            continue
                        P = pp.tile([128, S], BF16, name=f"P{qs}")
                        runs = []
                        for kc in vk:
                            if runs and runs[-1][1] == kc:
                                runs[-1][1] = kc + 1
                            else:
                                runs.append([kc, kc + 1])
                        sums = sp.tile([128, 4], F32, name="sums")
                        nsum = 0
                        for kc0, kc1 in runs:
                            for g0 in range(kc0, kc1, 2):
                                g1 = min(g0 + 2, kc1)
                                w = g1 - g0
                                ps = psp.tile([128, 1024], F32, name="mm1_ps",
                                              bufs=2)
                                for j, kc in enumerate(range(g0, g1)):
                                    sl = slice(j * 512, (j + 1) * 512)
                                    nc.tensor.matmul(
                                        ps[:, sl],
                                        qT[h][:, qt * 128:(qt + 1) * 128],
                                        kT[:, kc * 512:(kc + 1) * 512])
                                    tick()
                                    code = vis[qt][kc]
                                    if code >= 2:
                                        nc.vector.tensor_add(
                                            ps[:, sl], ps[:, sl],
                                            pat_sb[code - 2][:])
                                nc.scalar.activation(
                                    P[:, g0 * 512:g1 * 512],
                                    ps[:, 0:w * 512], EXP,
                                    scale=INV_SQRT_HD,
                                    accum_out=sums[:, nsum:nsum + 1])
                                nsum += 1
                        stot = sp.tile([128, 1], F32, name="stot")
                        if nsum > 1:
                            nc.vector.tensor_reduce(
                                stot[:], sums[:, 0:nsum],
                                axis=mybir.AxisListType.X, op=ADD)
                        else:
                            nc.vector.tensor_copy(stot[:], sums[:, 0:1])
                        recip = sp.tile([128, 1], F32, name="recip")
                        nc.vector.reciprocal(recip[:], stot[:])
                        for kc0, kc1 in runs:
                            seg = slice(kc0 * 512, kc1 * 512)
                            nc.vector.tensor_scalar_mul(P[:, seg], P[:, seg],
                                                        recip[:, 0:1])
                            nc.sync.dma_start_transpose(
                                PT[:, 4 * kc0:4 * kc1, qs * 128:(qs + 1) * 128],
                                P[:, seg])

                    def mm2(h=h, PT=PT):
                        ctx_ps = psp.tile([128, 512], F32, name="ctx_ps", bufs=2)
                        for j, kt in enumerate(kts):
                            nc.tensor.matmul(ctx_ps[:], v_all[:, kt, :],
                                             PT[:, kt, :], start=(j == 0),
                                             stop=(j == len(kts) - 1))
                            tick()
                        ctx_sb = cp.tile([128, 512], BF16, name=f"ctx{h}")
                        nc.vector.tensor_copy(ctx_sb[:], ctx_ps[:])
                        if split_ag:
                            ag_in, row = ag_ins[h // 2], (h % 2) * 128
                        else:
                            ag_in, row = ag_ins[0], h * 128
                        nc.sync.dma_start(ag_in[row:row + 128, :], ctx_sb[:])
                        done_mm2[0] += 1
                        if split_ag and done_mm2[0] == 2:
                            issue_ag(0)
                    mm2s.append(mm2)
                    if h >= 1:
                        mm2s.pop(0)()
                while mm2s:
                    mm2s.pop(0)()
                if split_ag:
                    issue_ag(1)
                else:
                    issue_ag(0)
                return ags

            # ---- pipelined emission ----
            f0 = Feeder()
            f0.add(qkv_gen(0))
            f0.drain()
            for i in range(n_pat):
                nc.scalar.dma_start(pat_sb[i][:], pats_d[i])
            for e in range(ET):
                nc.scalar.dma_start(wo_sb[e][:], woT_d[e * 128:(e + 1) * 128, :])

            fd = Feeder()
            fd.add(qkv_gen(1))
            fd.add(qkv_gen(2))
            ags0 = emit_attn(0, fd, 192)
            ctxF0 = oproj_start(0, ags0)
            fd.add(qkv_gen(3))
            fd.add(oproj_gen(0, ctxF0))
            ags1 = emit_attn(1, fd, 160)
            ctxF1 = oproj_start(1, ags1)
            fd.add(oproj_gen(1, ctxF1))
            ags2 = emit_attn(2, fd, 64)
            ctxF2 = oproj_start(2, ags2)
            fd.add(oproj_gen(2, ctxF2))
            ags3 = emit_attn(3, fd, 64, split_ag=True)
            ctxF3 = oproj_start(3, ags3)
            fd.add(oproj_gen(3, ctxF3,
                             iorder=[i for _, it in ags3 for i in it]))
            fd.drain()

    nc.compile()
    return nc


def kernel(hidden_states, wq, wk, wv, wo, attention_mask, position_ids):
    hidden_states = np.asarray(hidden_states, dtype=np.float32)
    wq = np.asarray(wq, dtype=np.float32)
    wk = np.asarray(wk, dtype=np.float32)
    wv = np.asarray(wv, dtype=np.float32)
    wo = np.asarray(wo, dtype=np.float32)
    pos = np.asarray(position_ids)

    vis, pats, n_pat = _classify_mask(attention_mask)
    key = (tuple(tuple(r) for r in vis), n_pat)
    if key not in _cache:
        _cache[key] = _build(vis, n_pat)
    nc = _cache[key]

    # RoPE tables per batch: cosT/sinT [HD, S]; sinT sign-folded (-sin for d<64)
    inv_freq = 1.0 / (ROPE_THETA ** (np.arange(0, HD, 2, dtype=np.float32) / HD))
    cosT = np.empty((B, HD, S), np.float32)
    sinT = np.empty((B, HD, S), np.float32)
    for b in range(B):
        freqs = pos[b].astype(np.float32)[None, :] * inv_freq[:, None]  # [64, S]
        cosT[b] = np.concatenate([np.cos(freqs)] * 2, axis=0)
        sn = np.sin(freqs)
        sinT[b] = np.concatenate([-sn, sn], axis=0)

    bf = ml_dtypes.bfloat16
    xT = [np.ascontiguousarray(hidden_states[b].T).astype(bf) for b in range(B)]
    in_maps = []
    for c in range(8):
        b, g = c // 4, c % 4
        in_maps.append({
            "xT": xT[b],
            "wqT": np.ascontiguousarray(wq[g * DL:(g + 1) * DL, :].T).astype(bf),
            "wkT": np.ascontiguousarray(wk[g * HD:(g + 1) * HD, :].T).astype(bf),
            "wvT": np.ascontiguousarray(wv[g * HD:(g + 1) * HD, :].T).astype(bf),
            "woT": np.ascontiguousarray(wo[g * DL:(g + 1) * DL, :].T).astype(bf),
            "cosT": cosT[b],
            "sinT": sinT[b],
            "pats": pats[b],
        })

    res = run_bass_kernel_spmd(nc, in_maps, core_ids=list(range(8))).results
    out = np.empty((B, S, H), np.float32)
    for c in range(8):
        b, g = c // 4, c % 4
        out[b, :, g * DL:(g + 1) * DL] = res[c]["out"]
    return out


# revision 18
# speedup vs baseline: 1.1442x; 1.0244x over previous
"""GQA attention layer (B=2, S=2048, H=2048, 16 q heads / 4 kv heads, RoPE, causal
mask) on 8 TRN2 NeuronCores.

Sharding: core c = (b, g) with b = c // 4 (batch), g = c % 4 (kv-head group).
Each core computes q-heads 4g..4g+3 and kv-head g for batch b:
  - QKV projections from x^T (bf16 matmuls, f32 PSUM accumulate)
  - RoPE on q/k (f32, host-precomputed cos/sin tables)
  - MM1 scores [q, k] -> mask -> fused exp+rowsum on ScalarE -> P (bf16)
  - P scaled by 1/rowsum, DMA-transposed -> P^T, MM2 -> ctx^T [d, q]
  - AllGather ctx^T across the 4 cores of the batch (groups [[0..3],[4..7]])
  - o-proj: each core computes its 512-column block of the output for all S.
Host reassembles [B, S, H] from the 8 [S, 512] column blocks.

The additive attention mask is handled generally: each 128x512 score tile is
classified at build time (from the actual mask input) as fully-masked (matmul
skipped), zero (no mask op), or mixed (a per-core mask pattern tile is added
pre-exp). For the causal mask this skips the upper triangle (~half the
attention FLOPs) and needs only 4 distinct pattern tiles.

Scheduling: the attention chain (MM1 -> exp on ScalarE -> scale -> DMA
transpose -> MM2) is latency- and ScalarE-bound, which starves TensorE and
triggers HAM re-throttling to half clock. So independent matmul work (QKV
projection of the next chunk, o-proj of the previous chunk) is fed into the
emission stream at single-matmul granularity between the dependent attention
matmuls, keeping TensorE dense. ScalarE runs exp exclusively (copies live on
VectorE) to avoid activation-table thrash.
"""
import math
import os
import sys

for _p in ("/opt/trn_rl_repo",):
    if _p not in sys.path and os.path.isdir(_p):
        sys.path.insert(0, _p)

import ml_dtypes
import numpy as np

from concourse import bacc, mybir, tile
from concourse.bass_utils import run_bass_kernel_spmd

BF16 = mybir.dt.bfloat16
F32 = mybir.dt.float32
EXP = mybir.ActivationFunctionType.Exp
ADD = mybir.AluOpType.add

B, S, H = 2, 2048, 2048
NH, NKV, HD = 16, 4, 128
GQ = NH // NKV            # q heads per core (4)
DL = GQ * HD              # local q width (512)
ET = H // 128             # e-tiles (16)
NSC = S // 512            # 512-wide s/k chunks (4)
NQT = S // 128            # 128-row q tiles (16)
NKT = S // 128            # 128-row k tiles (16)
ROPE_THETA = 10000.0
INV_SQRT_HD = 1.0 / math.sqrt(HD)
SQRT_HD = math.sqrt(HD)

SKIP, FREE = 0, 1         # vis codes; >=2 means pattern index (code - 2)

_cache = {}


def _classify_mask(mask):
    """mask: [B, 1, S, S] f32 additive. Returns (vis, pats) where
    vis[qt][kc] in {SKIP, FREE, idx+2} and pats[b] is [n_pat, 128, 512] f32
    (clipped, pre-multiplied by sqrt(HD))."""
    m = np.asarray(mask, dtype=np.float32).reshape(B, S, S)
    vis = [[FREE] * NSC for _ in range(NQT)]
    pat_ids = {}
    pats = [[] for _ in range(B)]
    for qt in range(NQT):
        for kc in range(NSC):
            blk = m[:, qt * 128:(qt + 1) * 128, kc * 512:(kc + 1) * 512]
            if np.all(blk <= -1e8):
                vis[qt][kc] = SKIP
            elif np.all(blk == 0.0):
                vis[qt][kc] = FREE
            else:
                clipped = np.maximum(blk, -90.0) * SQRT_HD
                key = clipped.tobytes()
                if key not in pat_ids:
                    pat_ids[key] = len(pats[0])
                    for b in range(B):
                        pats[b].append(clipped[b])
                vis[qt][kc] = pat_ids[key] + 2
    n_pat = len(pats[0])
    if n_pat == 0:
        pats_np = [np.zeros((1, 128, 512), np.float32) for _ in range(B)]
        n_pat = 1
    else:
        pats_np = [np.stack(p) for p in pats]
    return vis, pats_np, n_pat


class Feeder:
    """Round-robin pull of single-matmul filler steps from generators."""

    def __init__(self):
        self.gens = []

    def add(self, gen):
        self.gens.append(gen)

    def pull(self, n):
        while n > 0 and self.gens:
            try:
                next(self.gens[0])
                n -= 1
            except StopIteration:
                self.gens.pop(0)

    def drain(self):
        while self.gens:
            self.pull(1 << 30)


def _build(vis, n_pat):
    nc = bacc.Bacc(None, target_bir_lowering=False, num_devices=8)

    xT_d = nc.dram_tensor("xT", [H, S], BF16, kind="ExternalInput")
    wqT_d = nc.dram_tensor("wqT", [H, DL], BF16, kind="ExternalInput")
    wkT_d = nc.dram_tensor("wkT", [H, HD], BF16, kind="ExternalInput")
    wvT_d = nc.dram_tensor("wvT", [H, HD], BF16, kind="ExternalInput")
    woT_d = nc.dram_tensor("woT", [H, DL], BF16, kind="ExternalInput")
    cosT_d = nc.dram_tensor("cosT", [HD, S], F32, kind="ExternalInput")
    sinT_d = nc.dram_tensor("sinT", [HD, S], F32, kind="ExternalInput")
    pats_d = nc.dram_tensor("pats", [n_pat, 128, 512], F32, kind="ExternalInput")
    out_d = nc.dram_tensor("out", [S, DL], F32, kind="ExternalOutput")

    viskc = [[kc for kc in range(NSC) if vis[qt][kc] != SKIP] for qt in range(NQT)]
    vis_kts = []
    for c in range(NSC):
        kts = sorted({kt for qs in range(4) for kc in viskc[4 * c + qs]
                      for kt in range(4 * kc, 4 * kc + 4)})
        vis_kts.append(kts)
    uniform = all(
        all(viskc[4 * c + qs] == viskc[4 * c] for qs in range(4)) for c in range(NSC)
    )

    with tile.TileContext(nc) as tc:
        with (
            tc.tile_pool(name="wp", bufs=1) as wp,
            tc.tile_pool(name="xp", bufs=2) as xp,
            tc.tile_pool(name="qk", bufs=1) as qk,
            tc.tile_pool(name="rp", bufs=2) as rp,
            tc.tile_pool(name="pp", bufs=1) as pp,
            tc.tile_pool(name="ptp", bufs=2) as ptp,
            tc.tile_pool(name="cp", bufs=2) as cp,
            tc.tile_pool(name="fp", bufs=1) as fp,
            tc.tile_pool(name="op", bufs=2) as op,
            tc.tile_pool(name="sp", bufs=6) as sp,
            tc.tile_pool(name="ps", bufs=1, space="PSUM") as psp,
            tc.tile_pool(name="dr", bufs=2, space="DRAM") as dr,
        ):
            # ---- weights / tables for phase 1; wq/x interleaved and split
            # across both HWDGE trigger engines so the first projection
            # group's operands arrive first ----
            wq_sb = [wp.tile([128, DL], BF16, name=f"wq{e}") for e in range(ET)]
            wk_sb = [wp.tile([128, HD], BF16, name=f"wk{e}") for e in range(ET)]
            wv_sb = [wp.tile([128, HD], BF16, name=f"wv{e}") for e in range(ET)]
            for e in range(ET):
                r = slice(e * 128, (e + 1) * 128)
                nc.sync.dma_start(wq_sb[e][:], wqT_d[r, :])
            for e in range(ET):
                r = slice(e * 128, (e + 1) * 128)
                nc.scalar.dma_start(wk_sb[e][:], wkT_d[r, :])
                nc.scalar.dma_start(wv_sb[e][:], wvT_d[r, :])
            cos_sb = wp.tile([HD, S], F32, name="cos_sb")
            sin_sb = wp.tile([HD, S], F32, name="sin_sb")
            nc.scalar.dma_start(cos_sb[:], cosT_d[:])
            nc.scalar.dma_start(sin_sb[:], sinT_d[:])

            # ---- persistent activations ----
            qT = [qk.tile([HD, S], BF16, name=f"qT{h}") for h in range(GQ)]
            kT = qk.tile([HD, S], BF16, name="kT")
            v_all = qk.tile([128, NKT, HD], BF16, name="v_all")
            wo_sb = [wp.tile([128, DL], BF16, name=f"wo{e}") for e in range(ET)]
            pat_sb = [wp.tile([128, 512], F32, name=f"pat{i}") for i in range(n_pat)]

            def rope(ps, out_slice, sc):
                cs = slice(sc * 512, (sc + 1) * 512)
                t1 = rp.tile([128, 512], F32, name="rope_t1")
                nc.vector.tensor_mul(t1[0:64, :], ps[64:128, :], sin_sb[0:64, cs])
                nc.vector.tensor_mul(t1[64:128, :], ps[0:64, :], sin_sb[64:128, cs])
                t2 = rp.tile([128, 512], F32, name="rope_t2")
                nc.vector.tensor_mul(t2[:], ps[:], cos_sb[:, cs])
                nc.vector.tensor_add(out_slice, t2[:], t1[:])

            def qkv_gen(sc):
                """Yields once per matmul; 6 groups (4 q heads, k, v)."""
                cs = slice(sc * 512, (sc + 1) * 512)
                xts = []
                for e in range(ET):
                    t = xp.tile([128, 512], BF16, name=f"xts{e}")
                    nc.sync.dma_start(t[:], xT_d[e * 128:(e + 1) * 128, cs])
                    xts.append(t)

                def fin_q(h):
                    return lambda ps: rope(ps, qT[h][:, cs], sc)

                def fin_k(ps):
                    rope(ps, kT[:, cs], sc)

                def fin_v(ps):
                    vt = rp.tile([128, 512], BF16, name="vt_tmp")
                    nc.vector.tensor_copy(vt[:], ps[:])
                    nc.sync.dma_start_transpose(
                        v_all[:, sc * 4:(sc + 1) * 4, :], vt[:])

                blocks = [
                    (lambda e, h=h: wq_sb[e][:, h * 128:(h + 1) * 128], fin_q(h))
                    for h in range(2)
                ] + [
                    (lambda e: wk_sb[e][:], fin_k),
                    (lambda e: wv_sb[e][:], fin_v),
                ] + [
                    (lambda e, h=h: wq_sb[e][:, h * 128:(h + 1) * 128], fin_q(h))
                    for h in range(2, GQ)
                ]
                for lhs_fn, fin in blocks:
                    ps = psp.tile([128, 512], F32, name="fill_ps", bufs=2)
                    for e in range(ET):
                        nc.tensor.matmul(ps[:], lhs_fn(e), xts[e][:],
                                         start=(e == 0), stop=(e == ET - 1))
                        yield
                    fin(ps)

            def oproj_start(c, ag_outs):
                """Issue the gathered-ctx loads for o-proj chunk c on the
                GpSimd (SWDGE) queue, right after the AllGather in that
                queue's order — nothing else is blocked while they wait.
                Returns (tiles, order): tiles[g] is [128, 4, 512] covering
                4 global i-tiles; order is [(g, j, i), ...]."""
                tiles = []
                order = []
                g = 0
                for ag_out, itiles in ag_outs:
                    for r0 in range(0, len(itiles), 4):
                        t = fp.tile([128, 4, 512], BF16, name=f"ctxF{g}")
                        nc.gpsimd.dma_start(
                            t[:], ag_out[r0 * 128:(r0 + 4) * 128, :].rearrange(
                                "(a p) f -> p a f", p=128))
                        tiles.append(t)
                        for j in range(4):
                            order.append((g, j, itiles[r0 + j]))
                        g += 1
                return tiles, order

            def oproj_gen(c, ctx):
                """Yields once per matmul; 4 groups of 16 (o-proj chunk c)."""
                tiles, order = ctx
                for qs in range(4):
                    ops = psp.tile([128, 512], F32, name="fill_ps", bufs=2)
                    for j, (g, jj, i) in enumerate(order):
                        nc.tensor.matmul(
                            ops[:],
                            tiles[g][:, jj, qs * 128:(qs + 1) * 128],
                            wo_sb[i][:],
                            start=(j == 0), stop=(j == ET - 1))
                        yield
                    osb = op.tile([128, 512], F32, name="osb")
                    nc.vector.tensor_copy(osb[:], ops[:])
                    r0 = c * 512 + qs * 128
                    nc.gpsimd.dma_start(out_d[r0:r0 + 128, :], osb[:])

            def emit_attn(c, feeder, n_fill, split_ag=False):
                """Attention for q chunk c; pulls filler matmuls between the
                dependent stages to keep TensorE dense. Returns a list of
                (ag_out, itiles): the AllGather output dram tiles and the
                global i-tile (head*128 row-block) order inside each."""
                kts = vis_kts[c]
                # filler matmuls available per MM1/MM2 matmul emitted
                n_attn_mm = sum(len(viskc[4 * c + qs]) for qs in range(4)) * GQ * 2
                ratio = n_fill / max(1, n_attn_mm)
                acc = 0.0

                def tick(k=1):
                    nonlocal acc
                    acc += ratio * k
                    n = int(acc)
                    if n:
                        feeder.pull(n)
                        acc -= n

                if split_ag:
                    ag_ins = [dr.tile([2 * HD, 512], BF16, name="ag_in_a"),
                              dr.tile([2 * HD, 512], BF16, name="ag_in_b")]
                else:
                    ag_ins = [dr.tile([DL, 512], BF16, name="ag_in")]
                ags = []

                def issue_ag(part):
                    ag_in = ag_ins[part]
                    nr = ag_in.shape[0]
                    ag_out = dr.tile([4 * nr, 512], BF16,
                                     name=f"ag_out{'ab'[part] if split_ag else ''}")
                    nc.gpsimd.collective_compute(
                        "AllGather", mybir.AluOpType.bypass,
                        replica_groups=[[0, 1, 2, 3], [4, 5, 6, 7]],
                        ins=[ag_in[:].opt()], outs=[ag_out[:].opt()])
                    nh_part = nr // HD
                    itiles = [4 * r + part * nh_part + h
                              for r in range(4) for h in range(nh_part)]
                    ags.append((ag_out, itiles))

                done_mm2 = [0]
                mm2s = []
                for h in range(GQ):
                    PT = ptp.tile([128, NKT, 512], BF16, name="PT")
                    for qs in range(4):
                        qt = 4 * c + qs
                        vk = viskc[qt]
                        if not uniform:
                            for kt in kts:
                                if (kt // 4) not in vk:
                                    nc.vector.memset(
                                        PT[:, kt, qs * 128:(qs + 1) * 128], 0.0)
                        if not vk:
                            continue
                        P = pp.tile([128, S], BF16, name=f"P{qs}")
                        runs = []
                        for kc in vk:
                            if runs and runs[-1][1] == kc:
                                runs[-1][1] = kc + 1
                            else:
                                runs.append([kc, kc + 1])
                        sums = sp.tile([128, 4], F32, name="sums")
                        nsum = 0
                        for kc0, kc1 in runs:
                            for g0 in range(kc0, kc1, 2):
                                g1 = min(g0 + 2, kc1)
                                w = g1 - g0
                                ps = psp.tile([128, 1024], F32, name="mm1_ps",
                                              bufs=2)
                                for j, kc in enumerate(range(g0, g1)):
                                    sl = slice(j * 512, (j + 1) * 512)
                                    nc.tensor.matmul(
                                        ps[:, sl],
                                        qT[h][:, qt * 128:(qt + 1) * 128],
                                        kT[:, kc * 512:(kc + 1) * 512])
                                    tick()
                                    code = vis[qt][kc]
                                    if code >= 2:
                                        nc.vector.tensor_add(
                                            ps[:, sl], ps[:, sl],
                                            pat_sb[code - 2][:])
                                nc.scalar.activation(
                                    P[:, g0 * 512:g1 * 512],
                                    ps[:, 0:w * 512], EXP,
                                    scale=INV_SQRT_HD,
                                    accum_out=sums[:, nsum:nsum + 1])
                                nsum += 1
                        stot = sp.tile([128, 1], F32, name="stot")
                        if nsum > 1:
                            nc.vector.tensor_reduce(
                                stot[:], sums[:, 0:nsum],
                                axis=mybir.AxisListType.X, op=ADD)
                        else:
                            nc.vector.tensor_copy(stot[:], sums[:, 0:1])
                        recip = sp.tile([128, 1], F32, name="recip")
                        nc.vector.reciprocal(recip[:], stot[:])
                        for kc0, kc1 in runs:
                            seg = slice(kc0 * 512, kc1 * 512)
                            nc.vector.tensor_scalar_mul(P[:, seg], P[:, seg],
                                                        recip[:, 0:1])
                            nc.sync.dma_start_transpose(
                                PT[:, 4 * kc0:4 * kc1, qs * 128:(qs + 1) * 128],
                                P[:, seg])

                    def mm2(h=h, PT=PT):
                        ctx_ps = psp.tile([128, 512], F32, name="ctx_ps", bufs=2)
                        for j, kt in enumerate(kts):
                            nc.tensor.matmul(ctx_ps[:], v_all[:, kt, :],
                                             PT[:, kt, :], start=(j == 0),
                                             stop=(j == len(kts) - 1))
                            tick()
                        ctx_sb = cp.tile([128, 512], BF16, name=f"ctx{h}")
                        nc.vector.tensor_copy(ctx_sb[:], ctx_ps[:])
                        if split_ag:
                            ag_in, row = ag_ins[h // 2], (h % 2) * 128
                        else:
                            ag_in, row = ag_ins[0], h * 128
                        nc.sync.dma_start(ag_in[row:row + 128, :], ctx_sb[:])
                        done_mm2[0] += 1
                        if split_ag and done_mm2[0] == 2:
                            issue_ag(0)
                    mm2s.append(mm2)
                    if h >= 1:
                        mm2s.pop(0)()
                while mm2s:
                    mm2s.pop(0)()
                if split_ag:
                    issue_ag(1)
                else:
                    issue_ag(0)
                return ags

            # ---- pipelined emission ----
            f0 = Feeder()
            f0.add(qkv_gen(0))
            f0.drain()
            for i in range(n_pat):
                nc.scalar.dma_start(pat_sb[i][:], pats_d[i])
            for e in range(ET):
                nc.scalar.dma_start(wo_sb[e][:], woT_d[e * 128:(e + 1) * 128, :])

            fd = Feeder()
            fd.add(qkv_gen(1))
            fd.add(qkv_gen(2))
            ags0 = emit_attn(0, fd, 192)
            ctxF0 = oproj_start(0, ags0)
            fd.add(qkv_gen(3))
            ags1 = emit_attn(1, fd, 96)
            ctxF1 = oproj_start(1, ags1)
            fd.add(oproj_gen(0, ctxF0))
            ags2 = emit_attn(2, fd, 64)
            ctxF2 = oproj_start(2, ags2)
            fd.add(oproj_gen(1, ctxF1))
            ags3 = emit_attn(3, fd, 64, split_ag=True)
            ctxF3 = oproj_start(3, ags3)
            fd.add(oproj_gen(2, ctxF2))
            fd.add(oproj_gen(3, ctxF3))
            fd.drain()

    nc.compile()
    return nc


def kernel(hidden_states, wq, wk, wv, wo, attention_mask, position_ids):
    hidden_states = np.asarray(hidden_states, dtype=np.float32)
    wq = np.asarray(wq, dtype=np.float32)
    wk = np.asarray(wk, dtype=np.float32)
    wv = np.asarray(wv, dtype=np.float32)
    wo = np.asarray(wo, dtype=np.float32)
    pos = np.asarray(position_ids)

    vis, pats, n_pat = _classify_mask(attention_mask)
    key = (tuple(tuple(r) for r in vis), n_pat)
    if key not in _cache:
        _cache[key] = _build(vis, n_pat)
    nc = _cache[key]

    # RoPE tables per batch: cosT/sinT [HD, S]; sinT sign-folded (-sin for d<64)
    inv_freq = 1.0 / (ROPE_THETA ** (np.arange(0, HD, 2, dtype=np.float32) / HD))
    cosT = np.empty((B, HD, S), np.float32)
    sinT = np.empty((B, HD, S), np.float32)
    for b in range(B):
        freqs = pos[b].astype(np.float32)[None, :] * inv_freq[:, None]  # [64, S]
        cosT[b] = np.concatenate([np.cos(freqs)] * 2, axis=0)
        sn = np.sin(freqs)
        sinT[b] = np.concatenate([-sn, sn], axis=0)

    bf = ml_dtypes.bfloat16
    xT = [np.ascontiguousarray(hidden_states[b].T).astype(bf) for b in range(B)]
    in_maps = []
    for c in range(8):
        b, g = c // 4, c % 4
        in_maps.append({
            "xT": xT[b],
            "wqT": np.ascontiguousarray(wq[g * DL:(g + 1) * DL, :].T).astype(bf),
            "wkT": np.ascontiguousarray(wk[g * HD:(g + 1) * HD, :].T).astype(bf),
            "wvT": np.ascontiguousarray(wv[g * HD:(g + 1) * HD, :].T).astype(bf),
            "woT": np.ascontiguousarray(wo[g * DL:(g + 1) * DL, :].T).astype(bf),
            "cosT": cosT[b],
            "sinT": sinT[b],
            "pats": pats[b],
        })

    res = run_bass_kernel_spmd(nc, in_maps, core_ids=list(range(8))).results
    out = np.empty((B, S, H), np.float32)
    for c in range(8):
        b, g = c // 4, c % 4
        out[b, :, g * DL:(g + 1) * DL] = res[c]["out"]
    return out


# revision 22
# speedup vs baseline: 1.5508x; 1.3553x over previous
"""GQA attention layer (B=2, S=2048, H=2048, 16 q heads / 4 kv heads, RoPE, causal
mask) on 8 TRN2 NeuronCores.

Sharding: core c = (b, g) with b = c // 4 (batch), g = c % 4 (kv-head group).
Each core computes q-heads 4g..4g+3 and kv-head g for batch b:
  - QKV projections from x^T (bf16 matmuls, f32 PSUM accumulate)
  - RoPE on q/k (f32, host-precomputed cos/sin tables); v transposed via PE
  - MM1 computes TRANSPOSED scores S^T[k, q] (lhsT = k^T tile, rhs = q^T), so
    exp on ScalarE writes P^T directly -- no DMA transposes anywhere (Tile
    hard-serializes DMA transposes against collectives, which would freeze
    the pipeline during every AllGather)
  - row-sums of P via a ones-vector matmul (stationary never changes),
    1/sum applied to ctx^T with a partition-broadcast DMA + VectorE multiply
  - MM2: ctx^T[d, q] = sum_kt v[kt]^T-block @ P^T[kt]
  - AllGather ctx^T across the 4 cores of the batch (groups [[0..3],[4..7]])
  - o-proj: each core computes its 512-column block of the output for all S.
Host reassembles [B, S, H] from the 8 [S, 512] column blocks.

The additive attention mask is handled generally: each 512q x 128k score tile
is classified at build time (from the actual mask input) as fully-masked
(matmuls skipped), zero (no mask op), or mixed (a per-core transposed mask
pattern tile is added pre-exp). For the causal mask this skips the upper
triangle (~half the attention FLOPs) and needs only 4 distinct patterns.

Scheduling: the attention chain (MM1 -> exp -> sums -> MM2) is latency- and
ScalarE-bound; independent matmul work (QKV projection of later chunks,
o-proj of earlier chunks) is fed into the emission stream at single-matmul
granularity between the dependent attention matmuls, keeping TensorE dense.
AllGather-gated DMA (gathered-ctx loads, output writes) lives on the GpSimd
SWDGE queue so it cannot head-of-line-block compute-critical DMA queues.
"""
import math
import os
import sys

for _p in ("/opt/trn_rl_repo",):
    if _p not in sys.path and os.path.isdir(_p):
        sys.path.insert(0, _p)

import ml_dtypes
import numpy as np

from concourse import bacc, mybir, tile
from concourse.bass_utils import run_bass_kernel_spmd

BF16 = mybir.dt.bfloat16
F32 = mybir.dt.float32
EXP = mybir.ActivationFunctionType.Exp

B, S, H = 2, 2048, 2048
NH, NKV, HD = 16, 4, 128
GQ = NH // NKV            # q heads per core (4)
DL = GQ * HD              # local q width (512)
ET = H // 128             # e-tiles (16)
NSC = S // 512            # 512-wide s/q chunks (4)
NKT = S // 128            # 128-row k tiles (16)
ROPE_THETA = 10000.0
INV_SQRT_HD = 1.0 / math.sqrt(HD)
SQRT_HD = math.sqrt(HD)

SKIP, FREE = 0, 1         # vis codes; >=2 means pattern index (code - 2)

_cache = {}


def _classify_mask(mask):
    """mask: [B, 1, S, S] f32 additive. Tiles are [512 q x 128 k] per
    (chunk c, k-tile kt). Returns (vis, pats): vis[c][kt] in
    {SKIP, FREE, idx+2}; pats[b] is [n_pat, 128, 512] f32 — TRANSPOSED
    [k, q] patterns, clipped and pre-multiplied by sqrt(HD)."""
    m = np.asarray(mask, dtype=np.float32).reshape(B, S, S)
    vis = [[FREE] * NKT for _ in range(NSC)]
    pat_ids = {}
    pats = [[] for _ in range(B)]
    for c in range(NSC):
        for kt in range(NKT):
            blk = m[:, c * 512:(c + 1) * 512, kt * 128:(kt + 1) * 128]
            if np.all(blk <= -1e8):
                vis[c][kt] = SKIP
            elif np.all(blk == 0.0):
                vis[c][kt] = FREE
            else:
                clipped = np.maximum(blk, -90.0).transpose(0, 2, 1) * SQRT_HD
                clipped = np.ascontiguousarray(clipped)
                key = clipped.tobytes()
                if key not in pat_ids:
                    pat_ids[key] = len(pats[0])
                    for b in range(B):
                        pats[b].append(clipped[b])
                vis[c][kt] = pat_ids[key] + 2
    n_pat = len(pats[0])
    if n_pat == 0:
        pats_np = [np.zeros((1, 128, 512), np.float32) for _ in range(B)]
        n_pat = 1
    else:
        pats_np = [np.stack(p) for p in pats]
    return vis, pats_np, n_pat


class Feeder:
    """FIFO pull of single-matmul filler steps from generators."""

    def __init__(self):
        self.gens = []

    def add(self, gen):
        self.gens.append(gen)

    def pull(self, n):
        while n > 0 and self.gens:
            try:
                next(self.gens[0])
                n -= 1
            except StopIteration:
                self.gens.pop(0)

    def drain(self):
        while self.gens:
            self.pull(1 << 30)


def _build(vis, n_pat):
    nc = bacc.Bacc(None, target_bir_lowering=False, num_devices=8)

    xT_d = nc.dram_tensor("xT", [H, S], BF16, kind="ExternalInput")
    wqT_d = nc.dram_tensor("wqT", [H, DL], BF16, kind="ExternalInput")
    wkT_d = nc.dram_tensor("wkT", [H, HD], BF16, kind="ExternalInput")
    wvT_d = nc.dram_tensor("wvT", [H, HD], BF16, kind="ExternalInput")
    woT_d = nc.dram_tensor("woT", [H, DL], BF16, kind="ExternalInput")
    cosT_d = nc.dram_tensor("cosT", [HD, S], F32, kind="ExternalInput")
    sinT_d = nc.dram_tensor("sinT", [HD, S], F32, kind="ExternalInput")
    pats_d = nc.dram_tensor("pats", [n_pat, 128, 512], F32, kind="ExternalInput")
    ident_d = nc.dram_tensor("ident", [128, 128], BF16, kind="ExternalInput")
    out_d = nc.dram_tensor("out", [S, DL], F32, kind="ExternalOutput")

    vis_kts = [[kt for kt in range(NKT) if vis[c][kt] != SKIP] for c in range(NSC)]

    with tile.TileContext(nc) as tc:
        with (
            tc.tile_pool(name="wp", bufs=1) as wp,
            tc.tile_pool(name="xp", bufs=2) as xp,
            tc.tile_pool(name="qk", bufs=1) as qk,
            tc.tile_pool(name="rp", bufs=2) as rp,
            tc.tile_pool(name="ptp", bufs=2) as ptp,
            tc.tile_pool(name="cp", bufs=2) as cp,
            tc.tile_pool(name="fp", bufs=1) as fp,
            tc.tile_pool(name="op", bufs=2) as op,
            tc.tile_pool(name="sp", bufs=4) as sp,
            tc.tile_pool(name="ps", bufs=1, space="PSUM") as psp,
            tc.tile_pool(name="dr", bufs=2, space="DRAM") as dr,
        ):
            # ---- weights / tables for phase 1 ----
            wq_sb = [wp.tile([128, DL], BF16, name=f"wq{e}") for e in range(ET)]
            wk_sb = [wp.tile([128, HD], BF16, name=f"wk{e}") for e in range(ET)]
            wv_sb = [wp.tile([128, HD], BF16, name=f"wv{e}") for e in range(ET)]
            for e in range(ET):
                r = slice(e * 128, (e + 1) * 128)
                nc.sync.dma_start(wq_sb[e][:], wqT_d[r, :])
            for e in range(ET):
                r = slice(e * 128, (e + 1) * 128)
                nc.scalar.dma_start(wk_sb[e][:], wkT_d[r, :])
                nc.scalar.dma_start(wv_sb[e][:], wvT_d[r, :])
            cos_sb = wp.tile([HD, S], F32, name="cos_sb")
            sin_sb = wp.tile([HD, S], F32, name="sin_sb")
            nc.scalar.dma_start(cos_sb[:], cosT_d[:])
            nc.scalar.dma_start(sin_sb[:], sinT_d[:])
            ident_sb = wp.tile([128, 128], BF16, name="ident_sb")
            nc.scalar.dma_start(ident_sb[:], ident_d[:])
            ones_sb = wp.tile([128, 1], BF16, name="ones_sb")
            nc.vector.memset(ones_sb[:], 1.0)

            # ---- persistent activations ----
            qT = [qk.tile([HD, S], BF16, name=f"qT{h}") for h in range(GQ)]
            kT = qk.tile([HD, S], BF16, name="kT")
            v_all = qk.tile([128, NKT, HD], BF16, name="v_all")
            wo_sb = [wp.tile([128, DL], BF16, name=f"wo{e}") for e in range(ET)]
            pat_sb = [wp.tile([128, 512], F32, name=f"pat{i}") for i in range(n_pat)]

            def rope(ps, out_slice, sc):
                cs = slice(sc * 512, (sc + 1) * 512)
                t1 = rp.tile([128, 512], F32, name="rope_t1")
                nc.vector.tensor_mul(t1[0:64, :], ps[64:128, :], sin_sb[0:64, cs])
                nc.vector.tensor_mul(t1[64:128, :], ps[0:64, :], sin_sb[64:128, cs])
                t2 = rp.tile([128, 512], F32, name="rope_t2")
                nc.vector.tensor_mul(t2[:], ps[:], cos_sb[:, cs])
                nc.vector.tensor_add(out_slice, t2[:], t1[:])

            def qkv_gen(sc):
                """Yields once per matmul; 6 groups (4 q heads, k, v)."""
                cs = slice(sc * 512, (sc + 1) * 512)
                xts = []
                for e in range(ET):
                    t = xp.tile([128, 512], BF16, name=f"xts{e}")
                    nc.sync.dma_start(t[:], xT_d[e * 128:(e + 1) * 128, cs])
                    xts.append(t)

                def fin_q(h):
                    return lambda ps: rope(ps, qT[h][:, cs], sc)

                def fin_k(ps):
                    rope(ps, kT[:, cs], sc)

                def fin_v(ps):
                    vt = rp.tile([128, 512], BF16, name="vt_tmp")
                    nc.vector.tensor_copy(vt[:], ps[:])
                    for i in range(4):
                        vtp = psp.tile([128, 128], BF16, name="ctx_ps",
                                       tag="ctx_ps", bufs=2)
                        nc.tensor.transpose(vtp[:], vt[:, i * 128:(i + 1) * 128],
                                            ident_sb[:])
                        nc.vector.tensor_copy(v_all[:, sc * 4 + i, :], vtp[:])

                blocks = [
                    (lambda e, h=h: wq_sb[e][:, h * 128:(h + 1) * 128], fin_q(h))
                    for h in range(2)
                ] + [
                    (lambda e: wk_sb[e][:], fin_k),
                    (lambda e: wv_sb[e][:], fin_v),
                ] + [
                    (lambda e, h=h: wq_sb[e][:, h * 128:(h + 1) * 128], fin_q(h))
                    for h in range(2, GQ)
                ]
                for lhs_fn, fin in blocks:
                    ps = psp.tile([128, 512], F32, name="fill_ps", bufs=2)
                    for e in range(ET):
                        nc.tensor.matmul(ps[:], lhs_fn(e), xts[e][:],
                                         start=(e == 0), stop=(e == ET - 1))
                        yield
                    fin(ps)

            def oproj_start(c, ag_outs):
                """Issue the gathered-ctx loads for o-proj chunk c on the
                GpSimd (SWDGE) queue, right after the AllGather in that
                queue's order. Returns (tiles, order)."""
                tiles = []
                order = []
                g = 0
                for ag_out, itiles in ag_outs:
                    for r0 in range(0, len(itiles), 4):
                        t = fp.tile([128, 4, 512], BF16, name=f"ctxF{g}")
                        nc.gpsimd.dma_start(
                            t[:], ag_out[r0 * 128:(r0 + 4) * 128, :].rearrange(
                                "(a p) f -> p a f", p=128))
                        tiles.append(t)
                        for j in range(4):
                            order.append((g, j, itiles[r0 + j]))
                        g += 1
                return tiles, order

            def oproj_gen(c, ctx):
                """Yields once per matmul; 4 groups of 16 (o-proj chunk c)."""
                tiles, order = ctx
                for qs in range(4):
                    ops = psp.tile([128, 512], F32, name="fill_ps", bufs=2)
                    for j, (g, jj, i) in enumerate(order):
                        nc.tensor.matmul(
                            ops[:],
                            tiles[g][:, jj, qs * 128:(qs + 1) * 128],
                            wo_sb[i][:],
                            start=(j == 0), stop=(j == ET - 1))
                        yield
                    osb = op.tile([128, 512], F32, name="osb")
                    nc.vector.tensor_copy(osb[:], ops[:])
                    r0 = c * 512 + qs * 128
                    nc.gpsimd.dma_start(out_d[r0:r0 + 128, :], osb[:])

            def emit_attn(c, feeder, n_fill, split_ag=False):
                """Attention for q chunk c in transposed-score form."""
                kts = vis_kts[c]
                cs = slice(c * 512, (c + 1) * 512)
                n_attn_mm = len(kts) * GQ * 3  # MM1T + ones + MM2 per (h, kt)
                ratio = n_fill / max(1, n_attn_mm)
                acc = 0.0

                def tick(k=1):
                    nonlocal acc
                    acc += ratio * k
                    n = int(acc)
                    if n:
                        feeder.pull(n)
                        acc -= n

                if split_ag:
                    ag_ins = [dr.tile([2 * HD, 512], BF16, name="ag_in_a"),
                              dr.tile([2 * HD, 512], BF16, name="ag_in_b")]
                else:
                    ag_ins = [dr.tile([DL, 512], BF16, name="ag_in")]
                ags = []

                def issue_ag(part):
                    ag_in = ag_ins[part]
                    nr = ag_in.shape[0]
                    ag_out = dr.tile([4 * nr, 512], BF16,
                                     name=f"ag_out{'ab'[part] if split_ag else ''}")
                    nc.gpsimd.collective_compute(
                        "AllGather", mybir.AluOpType.bypass,
                        replica_groups=[[0, 1, 2, 3], [4, 5, 6, 7]],
                        ins=[ag_in[:].opt()], outs=[ag_out[:].opt()])
                    nh_part = nr // HD
                    itiles = [4 * r + part * nh_part + h
                              for r in range(4) for h in range(nh_part)]
                    ags.append((ag_out, itiles))

                for h in range(GQ):
                    PT = ptp.tile([128, NKT, 512], BF16, name="PT")
                    # MM1T + exp per visible k-tile
                    for kt in kts:
                        ps = psp.tile([128, 512], F32, name="mm1_ps", bufs=3)
                        nc.tensor.matmul(ps[:], kT[:, kt * 128:(kt + 1) * 128],
                                         qT[h][:, cs])
                        tick()
                        code = vis[c][kt]
                        if code >= 2:
                            nc.vector.tensor_add(ps[:], ps[:],
                                                 pat_sb[code - 2][:])
                        nc.scalar.activation(PT[:, kt, :], ps[:], EXP,
                                             scale=INV_SQRT_HD)
                    # row sums via ones-matmul (stationary constant)
                    sum_ps = psp.tile([1, 512], F32, name="sum_ps", bufs=1)
                    for j, kt in enumerate(kts):
                        nc.tensor.matmul(sum_ps[:], ones_sb[:], PT[:, kt, :],
                                         start=(j == 0), stop=(j == len(kts) - 1))
                        tick()
                    rrow = sp.tile([1, 512], F32, name="rrow")
                    nc.vector.reciprocal(rrow[:], sum_ps[:])
                    rbc = sp.tile([128, 512], F32, name="rbc")
                    nc.gpsimd.partition_broadcast(rbc[:], rrow[0:1, :])
                    # MM2
                    ctx_ps = psp.tile([128, 512], F32, name="ctx_ps",
                                      tag="ctx_ps", bufs=2)
                    for j, kt in enumerate(kts):
                        nc.tensor.matmul(ctx_ps[:], v_all[:, kt, :], PT[:, kt, :],
                                         start=(j == 0), stop=(j == len(kts) - 1))
                        tick()
                    ctx_sb = cp.tile([128, 512], BF16, name=f"ctx{h}")
                    nc.vector.tensor_mul(ctx_sb[:], ctx_ps[:], rbc[:])
                    if split_ag:
                        ag_in, row = ag_ins[h // 2], (h % 2) * 128
                    else:
                        ag_in, row = ag_ins[0], h * 128
                    nc.sync.dma_start(ag_in[row:row + 128, :], ctx_sb[:])
                    if split_ag and h == 1:
                        issue_ag(0)
                if split_ag:
                    issue_ag(1)
                else:
                    issue_ag(0)
                return ags

            # ---- pipelined emission ----
            f0 = Feeder()
            f0.add(qkv_gen(0))
            f0.drain()
            for i in range(n_pat):
                nc.scalar.dma_start(pat_sb[i][:], pats_d[i])
            for e in range(ET):
                nc.scalar.dma_start(wo_sb[e][:], woT_d[e * 128:(e + 1) * 128, :])

            fd = Feeder()
            fd.add(qkv_gen(1))
            fd.add(qkv_gen(2))
            ags0 = emit_attn(0, fd, 192)
            ctxF0 = oproj_start(0, ags0)
            fd.add(qkv_gen(3))
            ags1 = emit_attn(1, fd, 96)
            ctxF1 = oproj_start(1, ags1)
            fd.add(oproj_gen(0, ctxF0))
            ags2 = emit_attn(2, fd, 64)
            ctxF2 = oproj_start(2, ags2)
            fd.add(oproj_gen(1, ctxF1))
            ags3 = emit_attn(3, fd, 64, split_ag=True)
            ctxF3 = oproj_start(3, ags3)
            fd.add(oproj_gen(2, ctxF2))
            fd.add(oproj_gen(3, ctxF3))
            fd.drain()

    nc.compile()
    return nc


def _prepare(hidden_states, wq, wk, wv, wo, attention_mask, position_ids):
    hidden_states = np.asarray(hidden_states, dtype=np.float32)
    wq = np.asarray(wq, dtype=np.float32)
    wk = np.asarray(wk, dtype=np.float32)
    wv = np.asarray(wv, dtype=np.float32)
    wo = np.asarray(wo, dtype=np.float32)
    pos = np.asarray(position_ids)

    vis, pats, n_pat = _classify_mask(attention_mask)
    key = (tuple(tuple(r) for r in vis), n_pat)
    if key not in _cache:
        _cache[key] = _build(vis, n_pat)
    nc = _cache[key]

    # RoPE tables per batch: cosT/sinT [HD, S]; sinT sign-folded (-sin for d<64)
    inv_freq = 1.0 / (ROPE_THETA ** (np.arange(0, HD, 2, dtype=np.float32) / HD))
    cosT = np.empty((B, HD, S), np.float32)
    sinT = np.empty((B, HD, S), np.float32)
    for b in range(B):
        freqs = pos[b].astype(np.float32)[None, :] * inv_freq[:, None]  # [64, S]
        cosT[b] = np.concatenate([np.cos(freqs)] * 2, axis=0)
        sn = np.sin(freqs)
        sinT[b] = np.concatenate([-sn, sn], axis=0)

    bf = ml_dtypes.bfloat16
    xT = [np.ascontiguousarray(hidden_states[b].T).astype(bf) for b in range(B)]
    ident = np.eye(128, dtype=np.float32).astype(bf)
    in_maps = []
    for c in range(8):
        b, g = c // 4, c % 4
        in_maps.append({
            "xT": xT[b],
            "wqT": np.ascontiguousarray(wq[g * DL:(g + 1) * DL, :].T).astype(bf),
            "wkT": np.ascontiguousarray(wk[g * HD:(g + 1) * HD, :].T).astype(bf),
            "wvT": np.ascontiguousarray(wv[g * HD:(g + 1) * HD, :].T).astype(bf),
            "woT": np.ascontiguousarray(wo[g * DL:(g + 1) * DL, :].T).astype(bf),
            "cosT": cosT[b],
            "sinT": sinT[b],
            "pats": pats[b],
            "ident": ident,
        })
    return nc, in_maps


def kernel(hidden_states, wq, wk, wv, wo, attention_mask, position_ids):
    nc, in_maps = _prepare(hidden_states, wq, wk, wv, wo, attention_mask,
                           position_ids)
    res = run_bass_kernel_spmd(nc, in_maps, core_ids=list(range(8))).results
    out = np.empty((B, S, H), np.float32)
    for c in range(8):
        b, g = c // 4, c % 4
        out[b, :, g * DL:(g + 1) * DL] = res[c]["out"]
    return out
